# revision 1
# baseline (speedup 1.0000x reference)
"""Trainium2 Bass kernel for nn_Attention_60155311948227 (sparse_attention).

Sharding: data-parallel over batch B=8 across the 8 NeuronCores (1 sample per
core); the four FC weights are replicated (each core DMAs its own copy).

Per-core pipeline (GEMMs in bf16 with fp32 PSUM accumulation):
  XCT  = x_context^T   fp32 HWDGE row-strips -> PE transpose -> bf16
  A^T  = sum_{7x7}(x)  flat-layout loads, DVE reduce, PE transpose
  K^T  = BN(relu(kW @ xc^T + b))   [d1(part), m] bf16, kept in SBUF
  kn2  = ones-matmul of K^T**2 -> rk = 1/||k_row||;  qn2 -> rq
  S    = (Q^T)^T @ K^T  [n, m] * rq (row) * rk (col bcast) + amask, softmax
  P^T  = PE transpose;  P^T rows scaled by rv (V-row norms)
  V^T  -> PE transpose -> V_nat [m(part), d2] bf16 (unnormalized)
  WV^T = V_nat/P^T contraction; F^T = BN(relu(fW @ WV + b)) fp32
  out  = x + F broadcast over 7x7 (flat-layout passes, F via DRAM bounce)

Weights stream as fp32 column-strips on HWDGE and are cast to bf16 on-chip
(ACT/DVE) — the SWDGE cast-DMA path measures only ~45 GB/s aggregate.
"""

import sys

import numpy as np

try:
    import concourse.bacc as bacc
except ImportError:  # pragma: no cover
    sys.path.insert(0, "/opt/trn_rl_repo")
    import concourse.bacc as bacc

import ml_dtypes

import concourse.bass as bass
import concourse.tile as tile
from concourse import mybir
from concourse import bass_utils
from concourse.masks import make_identity

F32 = mybir.dt.float32
BF16 = mybir.dt.bfloat16
AF = mybir.ActivationFunctionType
ALU = mybir.AluOpType
AX = mybir.AxisListType

BN_EPS = 1e-5
NEG_MASK = -50.0
TEMP_INV = 100.0
NORM_EPS = 1e-24

FULL = dict(B=8, n=64, m=2048, D0=1024, C0=2048, D1=2048, D2=2048, KK=49)

P = 128


def build_program(cfg=None, num_devices=8):
    """Emit the SPMD per-core Bass program. Returns the compiled Bacc."""
    cfg = dict(FULL if cfg is None else cfg)
    n, m, D0, C0, D1, D2, KK = (
        cfg["n"], cfg["m"], cfg["D0"], cfg["C0"], cfg["D1"], cfg["D2"], cfg["KK"]
    )
    nc_d0, nc_c0, nc_d1, nc_d2, nc_m = D0 // P, C0 // P, D1 // P, D2 // P, m // P
    n_nt = max(1, m // 512)          # 512-wide moving-dim tiles
    NT = m // n_nt
    inv_kk = 1.0 / KK
    mh = m // 2
    # flat x/out chunking: partition p = (n, dhalf); per-partition contiguous
    DQ = 32                          # D-rows per flat chunk
    FD = DQ * KK                     # flat chunk free size
    NFC = (D0 // 2) // DQ            # number of flat chunks (8)

    nc = bacc.Bacc("TRN2", target_bir_lowering=False, debug=False,
                   num_devices=num_devices)

    def din(name, shape, dt=F32):
        return nc.dram_tensor(name, shape, dt, kind="ExternalInput").ap()

    x_in = din("x", [n, D0, KK])
    xc_in = din("xc", [m, C0])
    wqt = din("wqt", [D0, D1])
    wkt = din("wkt", [C0, D1])
    wvt = din("wvt", [C0, D2])
    wft = din("wft", [D2, D0])
    amask = din("amask", [m], BF16)
    qcb = din("qcb", [P, nc_d1]); qcg = din("qcg", [P, nc_d1]); qc2 = din("qc2", [P, nc_d1])
    kcb = din("kcb", [P, nc_d1]); kcg = din("kcg", [P, nc_d1]); kc2 = din("kc2", [P, nc_d1])
    vcb = din("vcb", [P, nc_d2]); vcg = din("vcg", [P, nc_d2]); vc2 = din("vc2", [P, nc_d2])
    fcb = din("fcb", [P, nc_d0]); fcg = din("fcg", [P, nc_d0]); fc2 = din("fc2", [P, nc_d0])
    out_d = nc.dram_tensor("out", [n, D0, KK], F32, kind="ExternalOutput").ap()
    x_flat = x_in.rearrange("nn d k -> (nn d k)").rearrange(
        "(p f) -> p f", p=P)          # [128, D0*KK/2] per-partition contiguous
    out_flat = out_d.rearrange("nn d k -> (nn d k)").rearrange(
        "(p f) -> p f", p=P)

    with tile.TileContext(nc) as tc:
        with (
            tc.tile_pool(name="consts", bufs=1) as consts,
            tc.tile_pool(name="bigmat", bufs=1) as bigmat,
            tc.tile_pool(name="w8", bufs=2) as w8,          # fp32 strips (8KB)
            tc.tile_pool(name="strips", bufs=3) as strips,  # bf16 strips (4KB)
            tc.tile_pool(name="smalls", bufs=2) as smalls,
            tc.tile_pool(name="wides", bufs=1) as wides,
            tc.tile_pool(name="xpool", bufs=2) as xpool,
            tc.tile_pool(name="ps", bufs=1, space="PSUM") as ps,
            tc.tile_pool(name="dscr", bufs=1, space="DRAM") as dscr,
        ):
            # ---------------- constants ----------------
            ident = consts.tile([P, P], BF16)
            make_identity(nc, ident)
            ident32 = consts.tile([P, P], F32)
            make_identity(nc, ident32)
            ones_col = consts.tile([P, 1], BF16)
            nc.vector.memset(ones_col, 1.0)
            eps_col = consts.tile([P, 1], F32)
            nc.vector.memset(eps_col, NORM_EPS)

            def cload(ap_in, nch):
                t = consts.tile([P, nch], F32, name=f"c_{ap_in.tensor.name}")
                nc.sync.dma_start(out=t, in_=ap_in)
                return t

            qcb_t = cload(qcb, nc_d1); qcg_t = cload(qcg, nc_d1); qc2_t = cload(qc2, nc_d1)
            kcb_t = cload(kcb, nc_d1); kcg_t = cload(kcg, nc_d1); kc2_t = cload(kc2, nc_d1)
            vcb_t = cload(vcb, nc_d2); vcg_t = cload(vcg, nc_d2); vc2_t = cload(vc2, nc_d2)
            fcb_t = cload(fcb, nc_d0); fcg_t = cload(fcg, nc_d0); fc2_t = cload(fc2, nc_d0)

            amask_bc = consts.tile([n, m], BF16, tag="amask_bc")
            nc.gpsimd.dma_start(
                out=amask_bc,
                in_=bass.AP(tensor=amask.tensor, offset=amask.offset,
                            ap=[[0, n]] + list(amask.ap)),
            )

            # ---------------- XCT: transpose x_context ----------------
            # contiguous fp32 row-strips; 16 fp32 PE transposes per strip into
            # an 8KB PSUM tile (alternating tag A/B); ACT copy casts to bf16.
            xct = bigmat.tile([P, nc_c0, m], BF16, tag="xct")
            for i in range(nc_m):
                xcs = w8.tile([P, C0], F32, tag="w8", name="xcs")
                nc.sync.dma_start(out=xcs, in_=xc_in[i * P:(i + 1) * P, :])
                tpx = ps.tile([P, nc_c0, P], F32,
                              tag=("A" if i % 2 == 0 else "B"), name="tpx")
                for c in range(nc_c0):
                    nc.tensor.transpose(tpx[:, c, :], xcs[:, c * P:(c + 1) * P],
                                        ident32)
                nc.scalar.copy(out=xct[:, :, i * P:(i + 1) * P], in_=tpx)

            # ---------------- pooling: A^T = sum_k x (flat layout) ----------
            at = consts.tile([P, nc_d0, n], BF16)
            for g in range(NFC):
                xt = xpool.tile([P, DQ, KK], F32, tag="x", name="xt")
                nc.sync.dma_start(out=xt,
                                  in_=x_flat[:, g * FD:(g + 1) * FD])
                asum = smalls.tile([P, DQ], F32, name="asum")
                nc.vector.reduce_sum(asum, xt, axis=AX.X)
                atp = ps.tile([DQ, P], F32, tag="B", name="atp")
                nc.tensor.transpose(atp, asum, ident32)
                # columns p=(nn, dhalf); D row = dhalf*D0/2 + g*DQ + dd2
                for half in range(2):
                    dglob = half * (D0 // 2) + g * DQ
                    base = dglob % P
                    nc.vector.tensor_copy(
                        out=at[base:base + DQ, dglob // P, :],
                        in_=atp[:, half::2])

            # ---------------- K^T projection (kept in SBUF) ----------------
            def wstrip(w_ap, j, ncc, name):
                """column-strip [P, ncc, P] bf16: even j via sync HWDGE fp32 +
                engine cast; odd j via SWDGE cast-DMA (parallel channel)."""
                wb = strips.tile([P, ncc, P], BF16, tag="strip", name=f"{name}b")
                src_ap = w_ap[:, j * P:(j + 1) * P].rearrange(
                    "(c p) w -> p c w", p=P)
                if j % 2 == 1:
                    nc.gpsimd.dma_start(out=wb, in_=src_ap)
                    return wb
                wf = w8.tile([P, ncc, P], F32, tag="w8", name=f"{name}f")
                nc.sync.dma_start(out=wf, in_=src_ap)
                nc.vector.tensor_copy(out=wb, in_=wf)
                return wb

            kt = bigmat.tile([P, nc_d1, m], BF16, tag="ktv", name="kt")
            kn2 = ps.tile([1, m], F32, tag="B")
            for j in range(nc_d1):
                kws = wstrip(wkt, j, nc_c0, "kws")
                kp = ps.tile([P, m], F32, tag="A", name="kp")
                for c in range(nc_c0):
                    for nt in range(n_nt):
                        nc.tensor.matmul(kp[:, nt * NT:(nt + 1) * NT],
                                         kws[:, c, :],
                                         xct[:, c, nt * NT:(nt + 1) * NT],
                                         start=(c == 0), stop=(c == nc_c0 - 1))
                ktj = kt[:, j, :]
                nc.scalar.activation(ktj[:, :mh], kp[:, :mh], AF.Relu,
                                     bias=kcb_t[:, j:j + 1])
                nc.vector.tensor_scalar(out=ktj[:, mh:], in0=kp[:, mh:],
                                        scalar1=kcb_t[:, j:j + 1], scalar2=0.0,
                                        op0=ALU.add, op1=ALU.max)
                nc.vector.tensor_scalar(out=ktj, in0=ktj,
                                        scalar1=kcg_t[:, j:j + 1],
                                        scalar2=kc2_t[:, j:j + 1],
                                        op0=ALU.mult, op1=ALU.add)
                ksq = w8.tile([P, m], BF16, tag="w8", name="ksq")
                nc.vector.tensor_mul(ksq, ktj, ktj)
                for nt in range(n_nt):
                    nc.tensor.matmul(kn2[:, nt * NT:(nt + 1) * NT], ones_col,
                                     ksq[:, nt * NT:(nt + 1) * NT],
                                     start=(j == 0), stop=(j == nc_d1 - 1))
            # rk chain: sqrt -> scatter [P, m/P] -> recip -> DRAM -> bcast
            rk_row = smalls.tile([1, m], F32, name="rk_row")
            nc.scalar.activation(rk_row, kn2, AF.Sqrt, bias=eps_col[:1, :])
            scr_k = dscr.tile([m], F32, name="scr_k")
            nc.gpsimd.dma_start(out=scr_k, in_=rk_row)
            rk128 = smalls.tile([P, nc_m], F32, name="rk128")
            nc.gpsimd.dma_start(out=rk128,
                                in_=bass.AP(tensor=scr_k.tensor, offset=scr_k.offset,
                                            ap=[[1, P], [P, nc_m]]))
            nc.vector.reciprocal(rk128, rk128)
            scr_k2 = dscr.tile([m], F32, name="scr_k2")
            nc.gpsimd.dma_start(
                out=bass.AP(tensor=scr_k2.tensor, offset=scr_k2.offset,
                            ap=[[1, P], [P, nc_m]]),
                in_=rk128)
            rk_bc = wides.tile([n, m], F32, name="rk_bc", tag="rk_bc")
            nc.gpsimd.dma_start(out=rk_bc,
                                in_=bass.AP(tensor=scr_k2.tensor, offset=scr_k2.offset,
                                            ap=[[0, n], [1, m]]))

            # ---------------- Q^T projection (c-incremental) ----------------
            # contiguous qwt row-strips; all 16 j-blocks accumulate in one
            # [P, nc_d1, n] PSUM tile across the 8 contraction chunks.
            qt = consts.tile([P, nc_d1, n], BF16)
            qps = ps.tile([P, nc_d1, n], F32, tag="B", name="qps")
            for c in range(nc_d0):
                qw8 = w8.tile([P, D1], F32, tag="w8", name="qw8")
                nc.sync.dma_start(out=qw8, in_=wqt[c * P:(c + 1) * P, :])
                qwb = strips.tile([P, D1], BF16, tag="strip", name="qwb")
                if c % 2 == 0:
                    nc.vector.tensor_copy(out=qwb, in_=qw8)
                else:
                    nc.scalar.copy(out=qwb, in_=qw8)
                for j in range(nc_d1):
                    # zero region = 2KB: j-blocks of n*4B; start only on the
                    # first matmul touching each region
                    jperz = max(1, 512 // n)
                    nc.tensor.matmul(qps[:, j, :], qwb[:, j * P:(j + 1) * P],
                                     at[:, c, :],
                                     start=(c == 0 and j % jperz == 0),
                                     stop=(c == nc_d0 - 1 and
                                           j % jperz == jperz - 1),
                                     skip_group_check=True)
            qn2 = ps.tile([1, n], F32, tag="A")
            for j in range(nc_d1):
                q1 = smalls.tile([P, n], BF16, name="q1")
                nc.scalar.activation(q1, qps[:, j, :], AF.Relu,
                                     bias=qcb_t[:, j:j + 1], scale=inv_kk)
                nc.vector.tensor_scalar(out=qt[:, j, :], in0=q1,
                                        scalar1=qcg_t[:, j:j + 1],
                                        scalar2=qc2_t[:, j:j + 1],
                                        op0=ALU.mult, op1=ALU.add)
                qsq = smalls.tile([P, n], BF16, name="qsq")
                nc.scalar.activation(qsq, qt[:, j, :], AF.Square)
                nc.tensor.matmul(qn2, ones_col, qsq,
                                 start=(j == 0), stop=(j == nc_d1 - 1))
            rq_row = smalls.tile([1, n], F32, name="rq_row")
            nc.scalar.activation(rq_row, qn2, AF.Sqrt, bias=eps_col[:1, :])
            scr_q = dscr.tile([n], F32, name="scr_q")
            nc.gpsimd.dma_start(out=scr_q, in_=rq_row)
            rq_col = smalls.tile([n, 1], F32, name="rq_col")
            nc.gpsimd.dma_start(out=rq_col,
                                in_=bass.AP(tensor=scr_q.tensor, offset=scr_q.offset,
                                            ap=[[1, n], [1, 1]]))
            nc.vector.reciprocal(rq_col, rq_col)

            # ---------------- S = Q K^T, softmax ----------------
            sp = ps.tile([n, m], F32, tag="B", name="sp")
            for j in range(nc_d1):
                for nt in range(n_nt):
                    nc.tensor.matmul(sp[:, nt * NT:(nt + 1) * NT], qt[:, j, :],
                                     kt[:, j, nt * NT:(nt + 1) * NT],
                                     start=(j == 0), stop=(j == nc_d1 - 1))
            nc.vector.tensor_scalar(out=sp, in0=sp, scalar1=rq_col,
                                    scalar2=None, op0=ALU.mult)
            nc.vector.tensor_mul(sp, sp, rk_bc)
            nc.vector.tensor_add(sp, sp, amask_bc)
            mxn = smalls.tile([n, 1], F32, name="mxn")
            nc.vector.tensor_reduce(mxn, sp, axis=AX.X, op=ALU.max, negate=True)
            ebias = smalls.tile([n, 1], F32, name="ebias")
            nc.vector.tensor_scalar_mul(ebias, mxn, TEMP_INV)
            p_t = consts.tile([n, m], BF16, name="p_t", tag="amask_bc")
            pden = smalls.tile([n, 1], F32, name="pden")
            nc.scalar.activation(p_t, sp, AF.Exp, bias=ebias, scale=TEMP_INV,
                                 accum_out=pden)
            nc.vector.reciprocal(pden, pden)
            nc.vector.tensor_scalar_mul(p_t, p_t, pden)
            ptp = ps.tile([P, nc_m, n], BF16, tag="B", name="ptp")
            for i in range(nc_m):
                nc.tensor.transpose(ptp[:, i, :], p_t[:, i * P:(i + 1) * P],
                                    ident[:n, :n])
            pt_sb = consts.tile([P, nc_m, n], BF16)
            nc.vector.tensor_copy(out=pt_sb, in_=ptp)

            # ---------------- V^T -> V_nat (unnormalized) ----------------
            v_nat = bigmat.tile([P, nc_m, D2], BF16, tag="ktv", name="v_nat")
            for j in range(nc_d2):
                vws = wstrip(wvt, j, nc_c0, "vws")
                vp = ps.tile([P, m], F32, tag="A", name="vp")
                for c in range(nc_c0):
                    for nt in range(n_nt):
                        nc.tensor.matmul(vp[:, nt * NT:(nt + 1) * NT],
                                         vws[:, c, :],
                                         xct[:, c, nt * NT:(nt + 1) * NT],
                                         start=(c == 0), stop=(c == nc_c0 - 1))
                vtj = strips.tile([P, m], BF16, tag="strip", name="vtj")
                nc.scalar.activation(vtj[:, :mh], vp[:, :mh], AF.Relu,
                                     bias=vcb_t[:, j:j + 1])
                nc.vector.tensor_scalar(out=vtj[:, mh:], in0=vp[:, mh:],
                                        scalar1=vcb_t[:, j:j + 1], scalar2=0.0,
                                        op0=ALU.add, op1=ALU.max)
                nc.vector.tensor_scalar(out=vtj, in0=vtj,
                                        scalar1=vcg_t[:, j:j + 1],
                                        scalar2=vc2_t[:, j:j + 1],
                                        op0=ALU.mult, op1=ALU.add)
                vtp = ps.tile([P, nc_m, P], BF16, tag="B", name="vtp")
                for i in range(nc_m):
                    nc.tensor.transpose(vtp[:, i, :], vtj[:, i * P:(i + 1) * P],
                                        ident)
                nc.vector.tensor_copy(out=v_nat[:, :, j * P:(j + 1) * P],
                                      in_=vtp)
            # rv = 1/||v_row||; folded into P^T rows (per-partition there)
            for i in range(nc_m):
                vsq = w8.tile([P, D2], BF16, tag="w8", name="vsq")
                vn2 = smalls.tile([P, 1], F32, name="vn2")
                nc.scalar.activation(vsq, v_nat[:, i, :], AF.Square,
                                     accum_out=vn2)
                rv = smalls.tile([P, 1], F32, name="rv")
                nc.scalar.activation(rv, vn2, AF.Sqrt, bias=eps_col)
                nc.vector.reciprocal(rv, rv)
                nc.vector.tensor_scalar_mul(pt_sb[:, i, :], pt_sb[:, i, :], rv)

            # ------------- WV^T and F^T fused over d2 chunks -------------
            # per d2-chunk j: WV_j = sum_i V_nat_i^T P^T_i, then immediately
            # accumulated into F via the j-th fwt row-strip (contiguous load).
            fps = ps.tile([P, nc_d0, n], F32, tag="B", name="fps")
            for j in range(nc_d2):
                fw8 = w8.tile([P, D0], F32, tag="w8", name="fw8")
                nc.sync.dma_start(out=fw8, in_=wft[j * P:(j + 1) * P, :])
                fwb = strips.tile([P, D0], BF16, tag="strip", name="fwb")
                if j % 2 == 0:
                    nc.vector.tensor_copy(out=fwb, in_=fw8)
                else:
                    nc.scalar.copy(out=fwb, in_=fw8)
                wvp = ps.tile([P, n], F32, tag="A", name="wvp")
                for i in range(nc_m):
                    nc.tensor.matmul(wvp, v_nat[:, i, j * P:(j + 1) * P],
                                     pt_sb[:, i, :],
                                     start=(i == 0), stop=(i == nc_m - 1))
                wvj = smalls.tile([P, n], BF16, name="wvj")
                nc.vector.tensor_copy(out=wvj, in_=wvp)
                ddperz = max(1, 512 // n)
                for dd in range(nc_d0):
                    nc.tensor.matmul(fps[:, dd, :], fwb[:, dd * P:(dd + 1) * P],
                                     wvj,
                                     start=(j == 0 and dd % ddperz == 0),
                                     stop=(j == nc_d2 - 1 and
                                           dd % ddperz == ddperz - 1),
                                     skip_group_check=True)
            ft = consts.tile([P, nc_d0, n], F32)
            for dd in range(nc_d0):
                f1 = smalls.tile([P, n], F32, name="f1")
                nc.scalar.activation(f1, fps[:, dd, :], AF.Relu,
                                     bias=fcb_t[:, dd:dd + 1])
                nc.vector.tensor_scalar(out=ft[:, dd, :], in0=f1,
                                        scalar1=fcg_t[:, dd:dd + 1],
                                        scalar2=fc2_t[:, dd:dd + 1],
                                        op0=ALU.mult, op1=ALU.add)

            # ---------------- out = x + F (flat layout) ----------------
            # F^T -> F_nat (PE transposes) -> DRAM bounce -> [(n dhalf), D0/2]
            fnat = wides.tile([n, D0], F32, tag="rk_bc")
            for dd in range(nc_d0):
                ftp = ps.tile([n, P], F32, tag="B", name="ftp")
                nc.tensor.transpose(ftp, ft[:, dd, :], ident32)
                nc.vector.tensor_copy(out=fnat[:, dd * P:(dd + 1) * P], in_=ftp)
            f_scr = dscr.tile([n, D0], F32, name="f_scr")
            nc.sync.dma_start(out=f_scr, in_=fnat)
            fperm = wides.tile([P, D0 // 2], F32, name="fperm", tag="rk_bc")
            nc.sync.dma_start(
                out=fperm,
                in_=bass.AP(tensor=f_scr.tensor, offset=f_scr.offset,
                            ap=[[D0, n], [D0 // 2, 2], [1, D0 // 2]]))
            for g in range(NFC):
                xo = xpool.tile([P, DQ, KK], F32, tag="x", name="xo")
                nc.sync.dma_start(out=xo, in_=x_flat[:, g * FD:(g + 1) * FD])
                nc.vector.tensor_add(
                    xo, xo,
                    fperm[:, g * DQ:(g + 1) * DQ].unsqueeze(2)
                    .broadcast_to([P, DQ, KK]))
                nc.scalar.dma_start(out=out_flat[:, g * FD:(g + 1) * FD], in_=xo)

    nc.compile()
    return nc


_CACHED = {}
# test-harness hook: extra kwargs for run_bass_kernel_spmd (e.g. trace=True)
_RUN_KWARGS = {}


def _get_program():
    if "nc" not in _CACHED:
        _CACHED["nc"] = build_program()
    return _CACHED["nc"]


def _bn_consts(b, gamma, beta, mean, var, nch):
    g = (gamma / np.sqrt(var + BN_EPS)).astype(np.float32)
    b2 = (beta - g * mean).astype(np.float32)
    def fold(v):
        return np.ascontiguousarray(np.asarray(v, np.float32).reshape(nch, P).T)
    return fold(b), fold(g), fold(b2)


def kernel(**inputs):
    cfg = FULL
    B, n, m = cfg["B"], cfg["n"], cfg["m"]
    D0, C0, D1, D2, KK = cfg["D0"], cfg["C0"], cfg["D1"], cfg["D2"], cfg["KK"]

    x = np.asarray(inputs["x"], dtype=np.float32).reshape(B, n, D0, KK)
    xc = np.asarray(inputs["x_context"], dtype=np.float32)
    nvalid = np.asarray(inputs["num_valid_context_items"]).reshape(B).astype(np.int64)

    wqt = np.ascontiguousarray(np.asarray(inputs["q_W"], np.float32).T)
    wkt = np.ascontiguousarray(np.asarray(inputs["k_W"], np.float32).T)
    wvt = np.ascontiguousarray(np.asarray(inputs["v_W"], np.float32).T)
    wft = np.ascontiguousarray(np.asarray(inputs["f_W"], np.float32).T)

    qc = _bn_consts(inputs["q_b"], inputs["q_gamma"], inputs["q_beta"],
                    inputs["q_mean"], inputs["q_var"], D1 // P)
    kc = _bn_consts(inputs["k_b"], inputs["k_gamma"], inputs["k_beta"],
                    inputs["k_mean"], inputs["k_var"], D1 // P)
    vc = _bn_consts(inputs["v_b"], inputs["v_gamma"], inputs["v_beta"],
                    inputs["v_mean"], inputs["v_var"], D2 // P)
    fc = _bn_consts(inputs["f_b"], inputs["f_gamma"], inputs["f_beta"],
                    inputs["f_mean"], inputs["f_var"], D0 // P)

    ar = np.arange(m)
    in_maps = []
    for b in range(B):
        am = np.where(ar < nvalid[b], 0.0, NEG_MASK).astype(ml_dtypes.bfloat16)
        in_maps.append({
            "x": np.ascontiguousarray(x[b]),
            "xc": np.ascontiguousarray(xc[b]),
            "wqt": wqt, "wkt": wkt, "wvt": wvt, "wft": wft,
            "amask": am,
            "qcb": qc[0], "qcg": qc[1], "qc2": qc[2],
            "kcb": kc[0], "kcg": kc[1], "kc2": kc[2],
            "vcb": vc[0], "vcg": vc[1], "vc2": vc[2],
            "fcb": fc[0], "fcg": fc[1], "fc2": fc[2],
        })

    nc = _get_program()
    res = bass_utils.run_bass_kernel_spmd(nc, in_maps, core_ids=list(range(B)),
                                          **_RUN_KWARGS)
    _CACHED["last_results"] = res
    out = np.stack([r["out"] for r in res.results], axis=0)
    return out.reshape(B, n, D0, 7, 7).astype(np.float32)



# revision 2
# speedup vs baseline: 1.0793x; 1.0793x over previous
"""Trainium2 Bass kernel for nn_Attention_60155311948227 (sparse_attention) v2.

Data-parallel over batch B=8 across 8 NeuronCores (1 sample/core); weights
replicated. Host-side prep: weights cast to bf16 (strip-swizzled for K/V),
x_context transposed+cast to bf16 on host -> no on-chip XCT phase.

Per-core emission order (PE executes its queue in order):
  pool    A^T = sum_{7x7}(x) (f32 SWDGE loads, DVE reduce, PE transpose)
  K^T     j-loop: kp halves rotate PSUM A/D; BN drains lag-1; ksq -> kn2
          ones-matmuls accumulate in persistent PSUM (C=lo, B=hi)
  rk      sqrt -> DRAM scatter -> recip -> bcast [n, m] (gpsimd, overlapped)
  Q^T     8 strips, qps in B; BN; qn2 ones-matmuls in C -> rq
  S       accumulated per j into spA (A) / spB (D); mask+softmax (DVE/ACT)
  V^T     j-loop like K (vp B/C); per-j PE transposes (vtp D) -> v_nat
          [m(part), d2] (aliases kt); rv2 accumulated on DVE per j
  WV/F    fused per d2-chunk j (wvp C/B), lag-2 F matmuls into fps (A)
  F       BN; F^T transposes (ftp D); DRAM bounce -> fperm
  out     x chunks (sync HWDGE) + F broadcast in-place add, store (scalar)
"""

import sys

import numpy as np

try:
    import concourse.bacc as bacc
except ImportError:  # pragma: no cover
    sys.path.insert(0, "/opt/trn_rl_repo")
    import concourse.bacc as bacc

import ml_dtypes

import concourse.bass as bass
import concourse.tile as tile
from concourse import mybir
from concourse import bass_utils
from concourse.masks import make_identity

F32 = mybir.dt.float32
BF16 = mybir.dt.bfloat16
AF = mybir.ActivationFunctionType
ALU = mybir.AluOpType
AX = mybir.AxisListType

BN_EPS = 1e-5
NEG_MASK = -50.0
TEMP_INV = 100.0
NORM_EPS = 1e-24

FULL = dict(B=8, n=64, m=2048, D0=1024, C0=2048, D1=2048, D2=2048, KK=49)

P = 128


def build_program(cfg=None, num_devices=8):
    cfg = dict(FULL if cfg is None else cfg)
    n, m, D0, C0, D1, D2, KK = (
        cfg["n"], cfg["m"], cfg["D0"], cfg["C0"], cfg["D1"], cfg["D2"], cfg["KK"]
    )
    nc_d0, nc_c0, nc_d1, nc_d2, nc_m = D0 // P, C0 // P, D1 // P, D2 // P, m // P
    NT = 512
    mh = m // 2
    inv_kk = 1.0 / KK
    DQ = 32
    FD = DQ * KK                    # 1568 elements per flat chunk

    nc = bacc.Bacc("TRN2", target_bir_lowering=False, debug=False,
                   num_devices=num_devices)

    def din(name, shape, dt=F32):
        return nc.dram_tensor(name, shape, dt, kind="ExternalInput").ap()

    x_in = din("x", [n, D0, KK])                     # f32
    xct_in = din("xct", [C0, m], BF16)               # host-transposed bf16
    wq_in = din("wq", [D0, D1], BF16)                # row strips
    wks_in = din("wks", [D1, C0], BF16)              # swizzled strips
    wvs_in = din("wvs", [D2, C0], BF16)              # swizzled strips
    wf_in = din("wf", [D2, D0], BF16)                # row strips
    amask = din("amask", [m], BF16)
    qcb = din("qcb", [P, nc_d1]); qcg = din("qcg", [P, nc_d1]); qc2 = din("qc2", [P, nc_d1])
    kcb = din("kcb", [P, nc_d1]); kcg = din("kcg", [P, nc_d1]); kc2 = din("kc2", [P, nc_d1])
    vcb = din("vcb", [P, nc_d2]); vcg = din("vcg", [P, nc_d2]); vc2 = din("vc2", [P, nc_d2])
    fcb = din("fcb", [P, nc_d0]); fcg = din("fcg", [P, nc_d0]); fc2 = din("fc2", [P, nc_d0])
    out_d = nc.dram_tensor("out", [n, D0, KK], F32, kind="ExternalOutput").ap()
    x_flat = x_in.rearrange("nn d k -> (nn d k)").rearrange("(p f) -> p f", p=P)
    out_flat = out_d.rearrange("nn d k -> (nn d k)").rearrange("(p f) -> p f", p=P)
    FREE = x_flat.shape[1]          # 25088
    NXC = FREE // FD                # 16 flat x chunks

    with tile.TileContext(nc) as tc:
        with (
            tc.tile_pool(name="consts", bufs=1) as consts,
            tc.tile_pool(name="bigmat", bufs=1) as bigmat,
            tc.tile_pool(name="wst", bufs=3) as wst,
            tc.tile_pool(name="vtjp", bufs=2) as vtjp,
            tc.tile_pool(name="scr", bufs=2) as scr,
            tc.tile_pool(name="smalls", bufs=2) as smalls,
            tc.tile_pool(name="xpool", bufs=3) as xpool,
            tc.tile_pool(name="ps", bufs=1, space="PSUM") as ps,
            tc.tile_pool(name="dscr", bufs=1, space="DRAM") as dscr,
        ):
            # ---------------- constants ----------------
            ident = consts.tile([P, P], BF16)
            make_identity(nc, ident)
            ident32 = consts.tile([P, P], F32)
            make_identity(nc, ident32)
            ones_col = consts.tile([P, 1], BF16)
            nc.vector.memset(ones_col, 1.0)
            eps_col = consts.tile([P, 1], F32)
            nc.vector.memset(eps_col, NORM_EPS)

            def cload(ap_in, nch):
                t = consts.tile([P, nch], F32, name=f"c_{ap_in.tensor.name}")
                nc.sync.dma_start(out=t, in_=ap_in)
                return t

            qcb_t = cload(qcb, nc_d1); qcg_t = cload(qcg, nc_d1); qc2_t = cload(qc2, nc_d1)
            kcb_t = cload(kcb, nc_d1); kcg_t = cload(kcg, nc_d1); kc2_t = cload(kc2, nc_d1)
            vcb_t = cload(vcb, nc_d2); vcg_t = cload(vcg, nc_d2); vc2_t = cload(vc2, nc_d2)
            fcb_t = cload(fcb, nc_d0); fcg_t = cload(fcg, nc_d0); fc2_t = cload(fc2, nc_d0)

            amask_bc = consts.tile([n, m], BF16, tag="amask_bc")
            nc.gpsimd.dma_start(
                out=amask_bc,
                in_=bass.AP(tensor=amask.tensor, offset=amask.offset,
                            ap=[[0, n]] + list(amask.ap)),
            )

            # ---------------- big SBUF tensors ----------------
            xct = bigmat.tile([P, nc_c0, m], BF16, tag="xct")
            nc.sync.dma_start(
                out=xct, in_=xct_in.rearrange("(c p) m -> p c m", p=P))

            kt = bigmat.tile([P, nc_d1, m], BF16, tag="ktv", name="kt")

            # ---------------- pooling: A^T = sum_k x (flat layout) ----------
            at = consts.tile([P, nc_d0, n], BF16)
            for g in range(NXC):
                xt = xpool.tile([P, FD], F32, tag="x", name="xt")
                nc.gpsimd.dma_start(out=xt, in_=x_flat[:, g * FD:(g + 1) * FD])
                asum = smalls.tile([P, DQ], F32, name="asum")
                nc.vector.reduce_sum(asum, xt.rearrange("p (d k) -> p d k", k=KK),
                                     axis=AX.X)
                atp = ps.tile([DQ, P], F32, tag="A", name="atp")
                nc.tensor.transpose(atp, asum, ident32)
                # partition p=(nn, dhalf); D row = dhalf*D0/2 + g*DQ + dd2
                for half in range(2):
                    dglob = half * (D0 // 2) + g * DQ
                    base = dglob % P
                    nc.vector.tensor_copy(
                        out=at[base:base + DQ, dglob // P, :],
                        in_=atp[:, half::2])

            # ---------------- K^T projection with fused kn2 ----------------
            kn2_lo = ps.tile([1, mh], F32, tag="C", name="kn2_lo")
            kn2_hi = ps.tile([1, mh], F32, tag="B", name="kn2_hi")

            def k_fill(j, h, kws):
                kp = ps.tile([P, mh], F32, tag=("A" if h == 0 else "D"),
                             name="kp")
                for c in range(nc_c0):
                    for nt in range(2):
                        lo = h * mh + nt * NT
                        nc.tensor.matmul(kp[:, nt * NT:(nt + 1) * NT],
                                         kws[:, c * P:(c + 1) * P],
                                         xct[:, c, lo:lo + NT],
                                         start=(c == 0), stop=(c == nc_c0 - 1))
                return kp

            def k_drain(j, h, kp):
                ktj = kt[:, j, h * mh:(h + 1) * mh]
                nc.scalar.activation(ktj, kp, AF.Relu, bias=kcb_t[:, j:j + 1])
                nc.vector.tensor_scalar(out=ktj, in0=ktj,
                                        scalar1=kcg_t[:, j:j + 1],
                                        scalar2=kc2_t[:, j:j + 1],
                                        op0=ALU.mult, op1=ALU.add)
                ksq = scr.tile([P, mh], BF16, tag="ksq", name="ksq")
                nc.vector.tensor_mul(ksq, ktj, ktj)
                kn2 = kn2_lo if h == 0 else kn2_hi
                for nt in range(2):
                    nc.tensor.matmul(kn2[:, nt * NT:(nt + 1) * NT], ones_col,
                                     ksq[:, nt * NT:(nt + 1) * NT],
                                     start=(j == 0), stop=(j == nc_d1 - 1))

            prev = None
            for j in range(nc_d1):
                kws = wst.tile([P, C0], BF16, tag="wst", name="kws")
                nc.sync.dma_start(out=kws, in_=wks_in[j * P:(j + 1) * P, :])
                for h in range(2):
                    kp = k_fill(j, h, kws)
                    if prev is not None:
                        k_drain(*prev)
                    prev = (j, h, kp)
            k_drain(*prev)

            # rk chain: sqrt -> scatter [P, m/P] -> recip -> DRAM -> bcast
            # rk_row shares the rk_bc region (consumed before rk_bc is written)
            rk_row = consts.tile([1, m], F32, name="rk_row", tag="rk_bc")
            nc.scalar.activation(rk_row[:, :mh], kn2_lo, AF.Sqrt,
                                 bias=eps_col[:1, :])
            nc.scalar.activation(rk_row[:, mh:], kn2_hi, AF.Sqrt,
                                 bias=eps_col[:1, :])
            scr_k = dscr.tile([m], F32, name="scr_k")
            nc.gpsimd.dma_start(out=scr_k, in_=rk_row)
            rk128 = smalls.tile([P, nc_m], F32, name="rk128")
            nc.gpsimd.dma_start(out=rk128,
                                in_=bass.AP(tensor=scr_k.tensor, offset=scr_k.offset,
                                            ap=[[1, P], [P, nc_m]]))
            nc.vector.reciprocal(rk128, rk128)
            scr_k2 = dscr.tile([m], F32, name="scr_k2")
            nc.gpsimd.dma_start(
                out=bass.AP(tensor=scr_k2.tensor, offset=scr_k2.offset,
                            ap=[[1, P], [P, nc_m]]),
                in_=rk128)
            rk_bc = consts.tile([n, m], F32, name="rk_bc", tag="rk_bc")
            nc.gpsimd.dma_start(out=rk_bc,
                                in_=bass.AP(tensor=scr_k2.tensor, offset=scr_k2.offset,
                                            ap=[[0, n], [1, m]]))

            # ---------------- Q^T projection + qn2 ----------------
            qt = consts.tile([P, nc_d1, n], BF16)
            qps = ps.tile([P, nc_d1, n], F32, tag="B", name="qps")
            jperz = max(1, 512 // n)
            for c in range(nc_d0):
                qwb = wst.tile([P, D1], BF16, tag="wst", name="qwb")
                nc.sync.dma_start(out=qwb, in_=wq_in[c * P:(c + 1) * P, :])
                for j in range(nc_d1):
                    nc.tensor.matmul(qps[:, j, :], qwb[:, j * P:(j + 1) * P],
                                     at[:, c, :],
                                     start=(c == 0 and j % jperz == 0),
                                     stop=(c == nc_d0 - 1 and
                                           j % jperz == jperz - 1),
                                     skip_group_check=True)
            qn2 = ps.tile([1, n], F32, tag="C")
            for j in range(nc_d1):
                q1 = smalls.tile([P, n], BF16, name="q1")
                nc.scalar.activation(q1, qps[:, j, :], AF.Relu,
                                     bias=qcb_t[:, j:j + 1], scale=inv_kk)
                nc.vector.tensor_scalar(out=qt[:, j, :], in0=q1,
                                        scalar1=qcg_t[:, j:j + 1],
                                        scalar2=qc2_t[:, j:j + 1],
                                        op0=ALU.mult, op1=ALU.add)
                qsq = smalls.tile([P, n], BF16, name="qsq")
                nc.scalar.activation(qsq, qt[:, j, :], AF.Square)
                nc.tensor.matmul(qn2, ones_col, qsq,
                                 start=(j == 0), stop=(j == nc_d1 - 1))
            rq_row = smalls.tile([1, n], F32, name="rq_row")
            nc.scalar.activation(rq_row, qn2, AF.Sqrt, bias=eps_col[:1, :])
            scr_q = dscr.tile([n], F32, name="scr_q")
            nc.gpsimd.dma_start(out=scr_q, in_=rq_row)
            rq_col = smalls.tile([n, 1], F32, name="rq_col")
            nc.gpsimd.dma_start(out=rq_col,
                                in_=bass.AP(tensor=scr_q.tensor, offset=scr_q.offset,
                                            ap=[[1, n], [1, 1]]))
            nc.vector.reciprocal(rq_col, rq_col)

            # ---------------- S = Q K^T (accumulated per j) ----------------
            spA = ps.tile([n, mh], F32, tag="A", name="spA")
            spB = ps.tile([n, mh], F32, tag="D", name="spB")
            for j in range(nc_d1):
                for h in range(2):
                    sp = spA if h == 0 else spB
                    for nt in range(2):
                        lo = h * mh + nt * NT
                        nc.tensor.matmul(sp[:, nt * NT:(nt + 1) * NT],
                                         qt[:, j, :], kt[:, j, lo:lo + NT],
                                         start=(j == 0), stop=(j == nc_d1 - 1))

            # ---------------- softmax ----------------
            mx = smalls.tile([n, 1], F32, name="mx")
            mxb = smalls.tile([n, 1], F32, name="mxb")
            p_t = consts.tile([n, m], BF16, name="p_t", tag="amask_bc")
            pden = smalls.tile([n, 1], F32, name="pden")
            pdenb = smalls.tile([n, 1], F32, name="pdenb")
            for h in range(2):
                sp = spA if h == 0 else spB
                nc.vector.tensor_scalar(out=sp, in0=sp, scalar1=rq_col,
                                        scalar2=None, op0=ALU.mult)
                nc.vector.tensor_mul(sp, sp, rk_bc[:, h * mh:(h + 1) * mh])
                nc.vector.tensor_add(sp, sp, amask_bc[:, h * mh:(h + 1) * mh])
                nc.vector.tensor_reduce(mx if h == 0 else mxb, sp, axis=AX.X,
                                        op=ALU.max, negate=True)
            nc.vector.tensor_tensor(out=mx, in0=mx, in1=mxb, op=ALU.min)
            ebias = smalls.tile([n, 1], F32, name="ebias")
            nc.vector.tensor_scalar_mul(ebias, mx, TEMP_INV)
            nc.scalar.activation(p_t[:, :mh], spA, AF.Exp, bias=ebias,
                                 scale=TEMP_INV, accum_out=pden)
            nc.scalar.activation(p_t[:, mh:], spB, AF.Exp, bias=ebias,
                                 scale=TEMP_INV, accum_out=pdenb)
            nc.vector.tensor_add(pden, pden, pdenb)
            nc.vector.reciprocal(pden, pden)
            nc.vector.tensor_scalar_mul(p_t, p_t, pden)

            # ---------------- V^T -> v_nat with fused rv2 ----------------
            v_nat = bigmat.tile([P, nc_m, D2], BF16, tag="ktv", name="v_nat")
            rv2 = consts.tile([P, nc_m], F32, name="rv2")
            nc.vector.memset(rv2, NORM_EPS)
            pt_sb = consts.tile([P, nc_m, n], BF16)

            def v_fill(j, h, vws):
                vp = ps.tile([P, mh], F32, tag=("B" if h == 0 else "C"),
                             name="vp")
                for c in range(nc_c0):
                    for nt in range(2):
                        lo = h * mh + nt * NT
                        nc.tensor.matmul(vp[:, nt * NT:(nt + 1) * NT],
                                         vws[:, c * P:(c + 1) * P],
                                         xct[:, c, lo:lo + NT],
                                         start=(c == 0), stop=(c == nc_c0 - 1))
                return vp

            def v_bn(j, h, vp, vtj):
                vtjh = vtj[:, h * mh:(h + 1) * mh]
                nc.scalar.activation(vtjh, vp, AF.Relu, bias=vcb_t[:, j:j + 1])
                nc.vector.tensor_scalar(out=vtjh, in0=vtjh,
                                        scalar1=vcg_t[:, j:j + 1],
                                        scalar2=vc2_t[:, j:j + 1],
                                        op0=ALU.mult, op1=ALU.add)

            def v_store(j, vtj):
                vtp = ps.tile([P, nc_m, P], BF16, tag="D", name="vtp")
                for i in range(nc_m):
                    nc.tensor.transpose(vtp[:, i, :], vtj[:, i * P:(i + 1) * P],
                                        ident)
                vslab = v_nat[:, :, j * P:(j + 1) * P]
                nc.vector.tensor_copy(out=vslab, in_=vtp)
                vsq = scr.tile([P, nc_m, P], BF16, tag="vsq", name="vsq", bufs=1)
                nc.vector.tensor_mul(vsq, vslab, vslab)
                vred = smalls.tile([P, nc_m], F32, name="vred")
                nc.vector.reduce_sum(vred, vsq, axis=AX.X)
                nc.vector.tensor_add(rv2, rv2, vred)

            pv = None
            for j in range(nc_d2):
                vws = wst.tile([P, C0], BF16, tag="wst", name="vws")
                nc.sync.dma_start(out=vws, in_=wvs_in[j * P:(j + 1) * P, :])
                vp0 = v_fill(j, 0, vws)
                if j == 1:
                    # P^T transposes: p_t ready (softmax ran during j=0 fills)
                    ptp = ps.tile([P, nc_m, n], BF16, tag="C", name="ptp")
                    for i in range(nc_m):
                        nc.tensor.transpose(ptp[:, i, :],
                                            p_t[:, i * P:(i + 1) * P],
                                            ident[:n, :n])
                    nc.vector.tensor_copy(out=pt_sb, in_=ptp)
                if pv is not None:
                    v_store(*pv)
                vp1 = v_fill(j, 1, vws)
                vtj = vtjp.tile([P, m], BF16, tag="vtj", name="vtj")
                v_bn(j, 0, vp0, vtj)
                v_bn(j, 1, vp1, vtj)
                pv = (j, vtj)
            v_store(*pv)

            # rv = rsqrt(rv2); scale P^T rows
            rv = smalls.tile([P, nc_m], F32, name="rv")
            nc.scalar.activation(rv, rv2, AF.Sqrt)
            nc.vector.reciprocal(rv, rv)
            for i in range(nc_m):
                nc.vector.tensor_scalar_mul(pt_sb[:, i, :], pt_sb[:, i, :],
                                            rv[:, i:i + 1])

            # ------------- WV^T and F^T fused over d2 chunks -------------
            fps = ps.tile([P, nc_d0, n], F32, tag="A", name="fps")
            ddperz = max(1, 512 // n)
            fstrips = {}

            def f_mm(j, wvj):
                for dd in range(nc_d0):
                    nc.tensor.matmul(fps[:, dd, :],
                                     fstrips[j][:, dd * P:(dd + 1) * P],
                                     wvj,
                                     start=(j == 0 and dd % ddperz == 0),
                                     stop=(j == nc_d2 - 1 and
                                           dd % ddperz == ddperz - 1),
                                     skip_group_check=True)

            wv_state = []
            for j in range(nc_d2):
                fwb = wst.tile([P, D0], BF16, tag="wst", name="fwb")
                nc.sync.dma_start(out=fwb, in_=wf_in[j * P:(j + 1) * P, :])
                fstrips[j] = fwb
                wvp = ps.tile([P, n], F32, tag=("C" if j % 2 == 0 else "B"),
                              name="wvp")
                for i in range(nc_m):
                    nc.tensor.matmul(wvp, v_nat[:, i, j * P:(j + 1) * P],
                                     pt_sb[:, i, :],
                                     start=(i == 0), stop=(i == nc_m - 1))
                wvj = smalls.tile([P, n], BF16, name="wvj", tag="wvj", bufs=3)
                nc.vector.tensor_copy(out=wvj, in_=wvp)
                wv_state.append((j, wvj))
                if len(wv_state) >= 3:
                    f_mm(*wv_state.pop(0))
            while wv_state:
                f_mm(*wv_state.pop(0))

            # ---------------- F BN, F^T -> flat layout ----------------
            ft = consts.tile([P, nc_d0, n], F32)
            for dd in range(nc_d0):
                f1 = smalls.tile([P, n], F32, name="f1")
                nc.scalar.activation(f1, fps[:, dd, :], AF.Relu,
                                     bias=fcb_t[:, dd:dd + 1])
                nc.vector.tensor_scalar(out=ft[:, dd, :], in0=f1,
                                        scalar1=fcg_t[:, dd:dd + 1],
                                        scalar2=fc2_t[:, dd:dd + 1],
                                        op0=ALU.mult, op1=ALU.add)
            fnat = consts.tile([n, D0], F32, tag="rk_bc")
            for dd in range(nc_d0):
                ftp = ps.tile([n, P], F32, tag="D", name="ftp")
                nc.tensor.transpose(ftp, ft[:, dd, :], ident32)
                nc.vector.tensor_copy(out=fnat[:, dd * P:(dd + 1) * P], in_=ftp)
            f_scr = dscr.tile([n, D0], F32, name="f_scr")
            nc.scalar.dma_start(out=f_scr, in_=fnat)
            fperm = consts.tile([P, D0 // 2], F32, name="fperm", tag="rk_bc")
            nc.scalar.dma_start(
                out=fperm,
                in_=bass.AP(tensor=f_scr.tensor, offset=f_scr.offset,
                            ap=[[D0, n], [D0 // 2, 2], [1, D0 // 2]]))

            # ---------------- out = x + F (flat layout) ----------------
            for g in range(NXC):
                xo = xpool.tile([P, FD], F32, tag="x", name="xo")
                nc.sync.dma_start(out=xo, in_=x_flat[:, g * FD:(g + 1) * FD])
                nc.vector.tensor_add(
                    xo.rearrange("p (d k) -> p d k", k=KK),
                    xo.rearrange("p (d k) -> p d k", k=KK),
                    fperm[:, g * DQ:(g + 1) * DQ].unsqueeze(2)
                    .broadcast_to([P, DQ, KK]))
                nc.scalar.dma_start(out=out_flat[:, g * FD:(g + 1) * FD], in_=xo)

    nc.compile()
    return nc


_CACHED = {}
_RUN_KWARGS = {}


def _get_program():
    if "nc" not in _CACHED:
        _CACHED["nc"] = build_program()
    return _CACHED["nc"]


def _bn_consts(b, gamma, beta, mean, var, nch):
    g = (gamma / np.sqrt(var + BN_EPS)).astype(np.float32)
    b2 = (beta - g * mean).astype(np.float32)
    def fold(v):
        return np.ascontiguousarray(np.asarray(v, np.float32).reshape(nch, P).T)
    return fold(b), fold(g), fold(b2)


def _swizzle(w, ncj, ncc):
    """w [J*128, C*128] -> out[j*128+p, c*128+ww] = w[j*128+ww, c*128+p]."""
    J, C = ncj, ncc
    a = np.asarray(w, np.float32).reshape(J, P, C, P)       # [j, ww, c, p]
    a = a.transpose(0, 3, 2, 1)                              # [j, p, c, ww]
    return np.ascontiguousarray(
        a.reshape(J * P, C * P).astype(ml_dtypes.bfloat16))


def kernel(**inputs):
    cfg = FULL
    B, n, m = cfg["B"], cfg["n"], cfg["m"]
    D0, C0, D1, D2, KK = cfg["D0"], cfg["C0"], cfg["D1"], cfg["D2"], cfg["KK"]

    x = np.asarray(inputs["x"], dtype=np.float32).reshape(B, n, D0, KK)
    xc = np.asarray(inputs["x_context"], dtype=np.float32)
    nvalid = np.asarray(inputs["num_valid_context_items"]).reshape(B).astype(np.int64)

    bf = ml_dtypes.bfloat16
    wq = np.ascontiguousarray(np.asarray(inputs["q_W"], np.float32).T.astype(bf))
    wks = _swizzle(inputs["k_W"], D1 // P, C0 // P)
    wvs = _swizzle(inputs["v_W"], D2 // P, C0 // P)
    wf = np.ascontiguousarray(np.asarray(inputs["f_W"], np.float32).T.astype(bf))

    qc = _bn_consts(inputs["q_b"], inputs["q_gamma"], inputs["q_beta"],
                    inputs["q_mean"], inputs["q_var"], D1 // P)
    kc = _bn_consts(inputs["k_b"], inputs["k_gamma"], inputs["k_beta"],
                    inputs["k_mean"], inputs["k_var"], D1 // P)
    vc = _bn_consts(inputs["v_b"], inputs["v_gamma"], inputs["v_beta"],
                    inputs["v_mean"], inputs["v_var"], D2 // P)
    fc = _bn_consts(inputs["f_b"], inputs["f_gamma"], inputs["f_beta"],
                    inputs["f_mean"], inputs["f_var"], D0 // P)

    ar = np.arange(m)
    in_maps = []
    for b in range(B):
        am = np.where(ar < nvalid[b], 0.0, NEG_MASK).astype(bf)
        in_maps.append({
            "x": np.ascontiguousarray(x[b]),
            "xct": np.ascontiguousarray(xc[b].T.astype(bf)),
            "wq": wq, "wks": wks, "wvs": wvs, "wf": wf,
            "amask": am,
            "qcb": qc[0], "qcg": qc[1], "qc2": qc[2],
            "kcb": kc[0], "kcg": kc[1], "kc2": kc[2],
            "vcb": vc[0], "vcg": vc[1], "vc2": vc[2],
            "fcb": fc[0], "fcg": fc[1], "fc2": fc[2],
        })

    nc = _get_program()
    res = bass_utils.run_bass_kernel_spmd(nc, in_maps, core_ids=list(range(B)),
                                          **_RUN_KWARGS)
    _CACHED["last_results"] = res
    out = np.stack([r["out"] for r in res.results], axis=0)
    return out.reshape(B, n, D0, 7, 7).astype(np.float32)


# revision 3
# speedup vs baseline: 1.2600x; 1.1675x over previous
"""Trainium2 Bass kernel for nn_Attention_60155311948227 (sparse_attention) v2.

Data-parallel over batch B=8 across 8 NeuronCores (1 sample/core); weights
replicated. Host-side prep: weights cast to bf16 (strip-swizzled for K/V),
x_context transposed+cast to bf16 on host -> no on-chip XCT phase.

Per-core emission order (PE executes its queue in order):
  pool    A^T = sum_{7x7}(x) (f32 SWDGE loads, DVE reduce, PE transpose)
  K^T     j-loop: kp halves rotate PSUM A/D; BN drains lag-1; ksq -> kn2
          ones-matmuls accumulate in persistent PSUM (C=lo, B=hi)
  rk      sqrt -> DRAM scatter -> recip -> bcast [n, m] (gpsimd, overlapped)
  Q^T     8 strips, qps in B; BN; qn2 ones-matmuls in C -> rq
  S       accumulated per j into spA (A) / spB (D); mask+softmax (DVE/ACT)
  V^T     j-loop like K (vp B/C); per-j PE transposes (vtp D) -> v_nat
          [m(part), d2] (aliases kt); rv2 accumulated on DVE per j
  WV/F    fused per d2-chunk j (wvp C/B), lag-2 F matmuls into fps (A)
  F       BN; F^T transposes (ftp D); DRAM bounce -> fperm
  out     x chunks (sync HWDGE) + F broadcast in-place add, store (scalar)
"""

import sys

import numpy as np

try:
    import concourse.bacc as bacc
except ImportError:  # pragma: no cover
    sys.path.insert(0, "/opt/trn_rl_repo")
    import concourse.bacc as bacc

import ml_dtypes

import concourse.bass as bass
import concourse.tile as tile
from concourse import mybir
from concourse import bass_utils
from concourse.masks import make_identity

F32 = mybir.dt.float32
BF16 = mybir.dt.bfloat16
AF = mybir.ActivationFunctionType
ALU = mybir.AluOpType
AX = mybir.AxisListType

BN_EPS = 1e-5
NEG_MASK = -50.0
TEMP_INV = 100.0
NORM_EPS = 1e-24

FULL = dict(B=8, n=64, m=2048, D0=1024, C0=2048, D1=2048, D2=2048, KK=49)

P = 128


def build_program(cfg=None, num_devices=8):
    cfg = dict(FULL if cfg is None else cfg)
    n, m, D0, C0, D1, D2, KK = (
        cfg["n"], cfg["m"], cfg["D0"], cfg["C0"], cfg["D1"], cfg["D2"], cfg["KK"]
    )
    nc_d0, nc_c0, nc_d1, nc_d2, nc_m = D0 // P, C0 // P, D1 // P, D2 // P, m // P
    NT = 512
    mh = m // 2
    inv_kk = 1.0 / KK
    DQ = 32
    FD = DQ * KK                    # 1568 elements per flat chunk

    nc = bacc.Bacc("TRN2", target_bir_lowering=False, debug=False,
                   num_devices=num_devices)

    def din(name, shape, dt=F32):
        return nc.dram_tensor(name, shape, dt, kind="ExternalInput").ap()

    xb_in = din("xb", [n, D0, KK], BF16)             # bf16 x (pool + residual)
    xct_in = din("xct", [C0, m], BF16)               # host-transposed bf16
    wq_in = din("wq", [D0, D1], BF16)                # row strips
    wks_in = din("wks", [D1, C0], BF16)              # swizzled strips
    wvs_in = din("wvs", [D2, C0], BF16)              # swizzled strips
    wf_in = din("wf", [D2, D0], BF16)                # row strips
    amask = din("amask", [m], BF16)
    qcb = din("qcb", [P, nc_d1]); qcg = din("qcg", [P, nc_d1]); qc2 = din("qc2", [P, nc_d1])
    kcb = din("kcb", [P, nc_d1]); kcg = din("kcg", [P, nc_d1]); kc2 = din("kc2", [P, nc_d1])
    vcb = din("vcb", [P, nc_d2]); vcg = din("vcg", [P, nc_d2]); vc2 = din("vc2", [P, nc_d2])
    fcb = din("fcb", [P, nc_d0]); fcg = din("fcg", [P, nc_d0]); fc2 = din("fc2", [P, nc_d0])
    out_d = nc.dram_tensor("out", [n, D0, KK], F32, kind="ExternalOutput").ap()
    xb_flat = xb_in.rearrange("nn d k -> (nn d k)").rearrange("(p f) -> p f", p=P)
    out_flat = out_d.rearrange("nn d k -> (nn d k)").rearrange("(p f) -> p f", p=P)
    FREE = xb_flat.shape[1]         # 25088
    NXC = FREE // FD                # 16 flat x chunks

    with tile.TileContext(nc) as tc:
        with (
            tc.tile_pool(name="consts", bufs=1) as consts,
            tc.tile_pool(name="bigmat", bufs=1) as bigmat,
            tc.tile_pool(name="wst", bufs=3) as wst,
            tc.tile_pool(name="vtjp", bufs=2) as vtjp,
            tc.tile_pool(name="scr", bufs=2) as scr,
            tc.tile_pool(name="smalls", bufs=2) as smalls,
            tc.tile_pool(name="xpool", bufs=3) as xpool,
            tc.tile_pool(name="ps", bufs=1, space="PSUM") as ps,
            tc.tile_pool(name="dscr", bufs=1, space="DRAM") as dscr,
        ):
            # ---------------- constants ----------------
            ident = consts.tile([P, P], BF16)
            make_identity(nc, ident)
            ident32 = consts.tile([P, P], F32)
            make_identity(nc, ident32)
            ones_col = consts.tile([P, 1], BF16)
            nc.vector.memset(ones_col, 1.0)
            eps_col = consts.tile([P, 1], F32)
            nc.vector.memset(eps_col, NORM_EPS)

            def cload(ap_in, nch):
                t = consts.tile([P, nch], F32, name=f"c_{ap_in.tensor.name}")
                nc.sync.dma_start(out=t, in_=ap_in)
                return t

            qcb_t = cload(qcb, nc_d1); qcg_t = cload(qcg, nc_d1); qc2_t = cload(qc2, nc_d1)
            kcb_t = cload(kcb, nc_d1); kcg_t = cload(kcg, nc_d1); kc2_t = cload(kc2, nc_d1)
            vcb_t = cload(vcb, nc_d2); vcg_t = cload(vcg, nc_d2); vc2_t = cload(vc2, nc_d2)
            fcb_t = cload(fcb, nc_d0); fcg_t = cload(fcg, nc_d0); fc2_t = cload(fc2, nc_d0)

            amask_bc = consts.tile([n, m], BF16, tag="amask_bc")
            nc.gpsimd.dma_start(
                out=amask_bc,
                in_=bass.AP(tensor=amask.tensor, offset=amask.offset,
                            ap=[[0, n]] + list(amask.ap)),
            )

            # ---------------- big SBUF tensors ----------------
            # xct in m-halves; first two K strips hoisted so K starts early
            xct = bigmat.tile([P, nc_c0, m], BF16, tag="xct")
            kws0 = wst.tile([P, C0], BF16, tag="wst", name="kws0")
            nc.sync.dma_start(out=kws0, in_=wks_in[0:P, :])
            xct_r = xct_in.rearrange("(c p) m -> p c m", p=P)
            nc.sync.dma_start(out=xct[:, :, :mh], in_=xct_r[:, :, :mh])
            kws1 = wst.tile([P, C0], BF16, tag="wst", name="kws1")
            nc.sync.dma_start(out=kws1, in_=wks_in[P:2 * P, :])
            nc.sync.dma_start(out=xct[:, :, mh:], in_=xct_r[:, :, mh:])

            kt = bigmat.tile([P, nc_d1, m], BF16, tag="ktv", name="kt")

            # pooling state: loads+reduces interleaved into the K loop below
            # asum_all[p, g, dd] = sum_k x[p, (g*DQ+dd)*KK + k]
            asum_all = consts.tile([P, NXC, DQ], F32, name="asum_all")
            at = consts.tile([P, nc_d0, n], BF16)

            def pool_chunk(g):
                xt = xpool.tile([P, FD], BF16, tag="x", name="xt")
                nc.sync.dma_start(out=xt, in_=xb_flat[:, g * FD:(g + 1) * FD])
                nc.vector.reduce_sum(asum_all[:, g, :],
                                     xt.rearrange("p (d k) -> p d k", k=KK),
                                     axis=AX.X)

            def pool_finish():
                # PE transposes (post-K, cheap): at[pp, c, nn] = A^T[cP+pp, nn]
                for g in range(NXC):
                    atp = ps.tile([DQ, P], F32, tag="A", name="atp")
                    nc.tensor.transpose(atp, asum_all[:, g, :], ident32)
                    for half in range(2):
                        dglob = half * (D0 // 2) + g * DQ
                        base = dglob % P
                        nc.vector.tensor_copy(
                            out=at[base:base + DQ, dglob // P, :],
                            in_=atp[:, half::2])

            # ---------------- K^T projection with fused kn2 ----------------
            kn2_lo = ps.tile([1, mh], F32, tag="C", name="kn2_lo")
            kn2_hi = ps.tile([1, mh], F32, tag="B", name="kn2_hi")

            def k_fill(j, h, kws):
                kp = ps.tile([P, mh], F32, tag=("A" if h == 0 else "D"),
                             name="kp")
                for c in range(nc_c0):
                    for nt in range(2):
                        lo = h * mh + nt * NT
                        nc.tensor.matmul(kp[:, nt * NT:(nt + 1) * NT],
                                         kws[:, c * P:(c + 1) * P],
                                         xct[:, c, lo:lo + NT],
                                         start=(c == 0), stop=(c == nc_c0 - 1))
                return kp

            def k_drain(j, h, kp):
                ktj = kt[:, j, h * mh:(h + 1) * mh]
                nc.scalar.activation(ktj, kp, AF.Relu, bias=kcb_t[:, j:j + 1])
                nc.vector.tensor_scalar(out=ktj, in0=ktj,
                                        scalar1=kcg_t[:, j:j + 1],
                                        scalar2=kc2_t[:, j:j + 1],
                                        op0=ALU.mult, op1=ALU.add)
                ksq = scr.tile([P, mh], BF16, tag="ksq", name="ksq")
                nc.vector.tensor_mul(ksq, ktj, ktj)
                kn2 = kn2_lo if h == 0 else kn2_hi
                for nt in range(2):
                    nc.tensor.matmul(kn2[:, nt * NT:(nt + 1) * NT], ones_col,
                                     ksq[:, nt * NT:(nt + 1) * NT],
                                     start=(j == 0), stop=(j == nc_d1 - 1))

            prev = None
            for j in range(nc_d1):
                if j == 0:
                    kws = kws0
                elif j == 1:
                    kws = kws1
                else:
                    kws = wst.tile([P, C0], BF16, tag="wst", name="kws")
                    nc.sync.dma_start(out=kws, in_=wks_in[j * P:(j + 1) * P, :])
                if j < NXC // 2:
                    pool_chunk(2 * j)
                    pool_chunk(2 * j + 1)
                for h in range(2):
                    kp = k_fill(j, h, kws)
                    if prev is not None:
                        k_drain(*prev)
                    prev = (j, h, kp)
            k_drain(*prev)
            pool_finish()

            # rk chain: sqrt -> scatter [P, m/P] -> recip -> DRAM -> bcast
            # rk_row shares the rk_bc region (consumed before rk_bc is written)
            rk_row = consts.tile([1, m], F32, name="rk_row", tag="rk_bc")
            nc.scalar.activation(rk_row[:, :mh], kn2_lo, AF.Sqrt,
                                 bias=eps_col[:1, :])
            nc.scalar.activation(rk_row[:, mh:], kn2_hi, AF.Sqrt,
                                 bias=eps_col[:1, :])
            scr_k = dscr.tile([m], F32, name="scr_k")
            nc.gpsimd.dma_start(out=scr_k, in_=rk_row)
            rk128 = smalls.tile([P, nc_m], F32, name="rk128")
            nc.gpsimd.dma_start(out=rk128,
                                in_=bass.AP(tensor=scr_k.tensor, offset=scr_k.offset,
                                            ap=[[1, P], [P, nc_m]]))
            nc.vector.reciprocal(rk128, rk128)
            scr_k2 = dscr.tile([m], F32, name="scr_k2")
            nc.gpsimd.dma_start(
                out=bass.AP(tensor=scr_k2.tensor, offset=scr_k2.offset,
                            ap=[[1, P], [P, nc_m]]),
                in_=rk128)
            rk_bc = consts.tile([n, m], F32, name="rk_bc", tag="rk_bc")
            nc.gpsimd.dma_start(out=rk_bc,
                                in_=bass.AP(tensor=scr_k2.tensor, offset=scr_k2.offset,
                                            ap=[[0, n], [1, m]]))

            # ---------------- Q^T projection + qn2 ----------------
            qt = consts.tile([P, nc_d1, n], BF16)
            qps = ps.tile([P, nc_d1, n], F32, tag="B", name="qps")
            jperz = max(1, 512 // n)
            for c in range(nc_d0):
                qwb = wst.tile([P, D1], BF16, tag="wst", name="qwb")
                nc.sync.dma_start(out=qwb, in_=wq_in[c * P:(c + 1) * P, :])
                for j in range(nc_d1):
                    nc.tensor.matmul(qps[:, j, :], qwb[:, j * P:(j + 1) * P],
                                     at[:, c, :],
                                     start=(c == 0 and j % jperz == 0),
                                     stop=(c == nc_d0 - 1 and
                                           j % jperz == jperz - 1),
                                     skip_group_check=True)
            qn2 = ps.tile([1, n], F32, tag="C")
            for j in range(nc_d1):
                q1 = smalls.tile([P, n], BF16, name="q1")
                nc.scalar.activation(q1, qps[:, j, :], AF.Relu,
                                     bias=qcb_t[:, j:j + 1], scale=inv_kk)
                nc.vector.tensor_scalar(out=qt[:, j, :], in0=q1,
                                        scalar1=qcg_t[:, j:j + 1],
                                        scalar2=qc2_t[:, j:j + 1],
                                        op0=ALU.mult, op1=ALU.add)
                qsq = smalls.tile([P, n], BF16, name="qsq")
                nc.scalar.activation(qsq, qt[:, j, :], AF.Square)
                nc.tensor.matmul(qn2, ones_col, qsq,
                                 start=(j == 0), stop=(j == nc_d1 - 1))
            rq_row = smalls.tile([1, n], F32, name="rq_row")
            nc.scalar.activation(rq_row, qn2, AF.Sqrt, bias=eps_col[:1, :])
            scr_q = dscr.tile([n], F32, name="scr_q")
            nc.gpsimd.dma_start(out=scr_q, in_=rq_row)
            rq_col = smalls.tile([n, 1], F32, name="rq_col")
            nc.gpsimd.dma_start(out=rq_col,
                                in_=bass.AP(tensor=scr_q.tensor, offset=scr_q.offset,
                                            ap=[[1, n], [1, 1]]))
            nc.vector.reciprocal(rq_col, rq_col)

            # ---------------- S = Q K^T (accumulated per j) ----------------
            spA = ps.tile([n, mh], F32, tag="A", name="spA")
            spB = ps.tile([n, mh], F32, tag="D", name="spB")
            for j in range(nc_d1):
                for h in range(2):
                    sp = spA if h == 0 else spB
                    for nt in range(2):
                        lo = h * mh + nt * NT
                        nc.tensor.matmul(sp[:, nt * NT:(nt + 1) * NT],
                                         qt[:, j, :], kt[:, j, lo:lo + NT],
                                         start=(j == 0), stop=(j == nc_d1 - 1))

            # ---------------- softmax ----------------
            mx = smalls.tile([n, 1], F32, name="mx")
            mxb = smalls.tile([n, 1], F32, name="mxb")
            p_t = consts.tile([n, m], BF16, name="p_t", tag="amask_bc")
            pden = smalls.tile([n, 1], F32, name="pden")
            pdenb = smalls.tile([n, 1], F32, name="pdenb")
            for h in range(2):
                sp = spA if h == 0 else spB
                nc.vector.tensor_scalar(out=sp, in0=sp, scalar1=rq_col,
                                        scalar2=None, op0=ALU.mult)
                nc.vector.tensor_mul(sp, sp, rk_bc[:, h * mh:(h + 1) * mh])
                nc.vector.tensor_add(sp, sp, amask_bc[:, h * mh:(h + 1) * mh])
                nc.vector.tensor_reduce(mx if h == 0 else mxb, sp, axis=AX.X,
                                        op=ALU.max, negate=True)
            nc.vector.tensor_tensor(out=mx, in0=mx, in1=mxb, op=ALU.min)
            ebias = smalls.tile([n, 1], F32, name="ebias")
            nc.vector.tensor_scalar_mul(ebias, mx, TEMP_INV)
            nc.scalar.activation(p_t[:, :mh], spA, AF.Exp, bias=ebias,
                                 scale=TEMP_INV, accum_out=pden)
            nc.scalar.activation(p_t[:, mh:], spB, AF.Exp, bias=ebias,
                                 scale=TEMP_INV, accum_out=pdenb)
            nc.vector.tensor_add(pden, pden, pdenb)
            nc.vector.reciprocal(pden, pden)
            nc.vector.tensor_scalar_mul(p_t, p_t, pden)

            # ---------------- V^T -> v_nat with fused rv2 ----------------
            v_nat = bigmat.tile([P, nc_m, D2], BF16, tag="ktv", name="v_nat")
            rv2 = consts.tile([P, nc_m], F32, name="rv2")
            nc.vector.memset(rv2, NORM_EPS)
            pt_sb = consts.tile([P, nc_m, n], BF16)

            def v_fill(j, h, vws):
                vp = ps.tile([P, mh], F32, tag=("B" if h == 0 else "C"),
                             name="vp")
                for c in range(nc_c0):
                    for nt in range(2):
                        lo = h * mh + nt * NT
                        nc.tensor.matmul(vp[:, nt * NT:(nt + 1) * NT],
                                         vws[:, c * P:(c + 1) * P],
                                         xct[:, c, lo:lo + NT],
                                         start=(c == 0), stop=(c == nc_c0 - 1))
                return vp

            def v_bn(j, h, vp, vtj):
                vtjh = vtj[:, h * mh:(h + 1) * mh]
                nc.scalar.activation(vtjh, vp, AF.Relu, bias=vcb_t[:, j:j + 1])
                nc.vector.tensor_scalar(out=vtjh, in0=vtjh,
                                        scalar1=vcg_t[:, j:j + 1],
                                        scalar2=vc2_t[:, j:j + 1],
                                        op0=ALU.mult, op1=ALU.add)

            def v_store(j, vtj):
                vtp = ps.tile([P, nc_m, P], BF16, tag="D", name="vtp")
                for i in range(nc_m):
                    nc.tensor.transpose(vtp[:, i, :], vtj[:, i * P:(i + 1) * P],
                                        ident)
                vslab = v_nat[:, :, j * P:(j + 1) * P]
                nc.vector.tensor_copy(out=vslab, in_=vtp)
                vsq = scr.tile([P, nc_m, P], BF16, tag="vsq", name="vsq", bufs=1)
                nc.vector.tensor_mul(vsq, vslab, vslab)
                vred = smalls.tile([P, nc_m], F32, name="vred")
                nc.vector.reduce_sum(vred, vsq, axis=AX.X)
                nc.vector.tensor_add(rv2, rv2, vred)

            pv = None
            for j in range(nc_d2):
                vws = wst.tile([P, C0], BF16, tag="wst", name="vws")
                nc.sync.dma_start(out=vws, in_=wvs_in[j * P:(j + 1) * P, :])
                vp0 = v_fill(j, 0, vws)
                if j == 1:
                    # P^T transposes: p_t ready (softmax ran during j=0 fills)
                    ptp = ps.tile([P, nc_m, n], BF16, tag="C", name="ptp")
                    for i in range(nc_m):
                        nc.tensor.transpose(ptp[:, i, :],
                                            p_t[:, i * P:(i + 1) * P],
                                            ident[:n, :n])
                    nc.vector.tensor_copy(out=pt_sb, in_=ptp)
                if pv is not None:
                    v_store(*pv)
                vp1 = v_fill(j, 1, vws)
                vtj = vtjp.tile([P, m], BF16, tag="vtj", name="vtj")
                v_bn(j, 0, vp0, vtj)
                v_bn(j, 1, vp1, vtj)
                pv = (j, vtj)
            v_store(*pv)

            # x_res prefetch into the xct region (dead after V proj)
            x_res = bigmat.tile([P, FREE], BF16, tag="xct", name="x_res")
            nc.gpsimd.dma_start(out=x_res, in_=xb_flat)

            # rv = rsqrt(rv2); scale P^T rows (broadcast + single mul)
            rv = smalls.tile([P, nc_m], F32, name="rv")
            nc.scalar.activation(rv, rv2, AF.Sqrt)
            nc.vector.reciprocal(rv, rv)
            rv_bc = scr.tile([P, nc_m, n], F32, tag="vsq", name="rv_bc", bufs=1)
            nc.vector.tensor_copy(out=rv_bc,
                                  in_=rv.unsqueeze(2).broadcast_to([P, nc_m, n]))
            nc.vector.tensor_mul(pt_sb, pt_sb, rv_bc)

            # ------------- WV^T and F^T fused over d2 chunks -------------
            fps = ps.tile([P, nc_d0, n], F32, tag="A", name="fps")
            ddperz = max(1, 512 // n)
            fstrips = {}

            def f_mm(j, wvj):
                for dd in range(nc_d0):
                    nc.tensor.matmul(fps[:, dd, :],
                                     fstrips[j][:, dd * P:(dd + 1) * P],
                                     wvj,
                                     start=(j == 0 and dd % ddperz == 0),
                                     stop=(j == nc_d2 - 1 and
                                           dd % ddperz == ddperz - 1),
                                     skip_group_check=True)

            wv_state = []
            for j in range(nc_d2):
                fwb = wst.tile([P, D0], BF16, tag="wst", name="fwb")
                nc.sync.dma_start(out=fwb, in_=wf_in[j * P:(j + 1) * P, :])
                fstrips[j] = fwb
                wvp = ps.tile([P, n], F32, tag=("C" if j % 2 == 0 else "B"),
                              name="wvp")
                for i in range(nc_m):
                    nc.tensor.matmul(wvp, v_nat[:, i, j * P:(j + 1) * P],
                                     pt_sb[:, i, :],
                                     start=(i == 0), stop=(i == nc_m - 1))
                wvj = smalls.tile([P, n], BF16, name="wvj", tag="wvj", bufs=3)
                nc.vector.tensor_copy(out=wvj, in_=wvp)
                wv_state.append((j, wvj))
                if len(wv_state) >= 3:
                    f_mm(*wv_state.pop(0))
            while wv_state:
                f_mm(*wv_state.pop(0))

            # ---------------- F BN, F^T -> flat layout ----------------
            ft = consts.tile([P, nc_d0, n], F32)
            for dd in range(nc_d0):
                f1 = smalls.tile([P, n], F32, name="f1")
                nc.scalar.activation(f1, fps[:, dd, :], AF.Relu,
                                     bias=fcb_t[:, dd:dd + 1])
                nc.vector.tensor_scalar(out=ft[:, dd, :], in0=f1,
                                        scalar1=fcg_t[:, dd:dd + 1],
                                        scalar2=fc2_t[:, dd:dd + 1],
                                        op0=ALU.mult, op1=ALU.add)
            fnat = consts.tile([n, D0], F32, tag="rk_bc")
            for dd in range(nc_d0):
                ftp = ps.tile([n, P], F32, tag="D", name="ftp")
                nc.tensor.transpose(ftp, ft[:, dd, :], ident32)
                nc.vector.tensor_copy(out=fnat[:, dd * P:(dd + 1) * P], in_=ftp)
            f_scr = dscr.tile([n, D0], F32, name="f_scr")
            nc.scalar.dma_start(out=f_scr, in_=fnat)
            fperm = consts.tile([P, D0 // 2], BF16, name="fperm")
            nc.gpsimd.dma_start(
                out=fperm,
                in_=bass.AP(tensor=f_scr.tensor, offset=f_scr.offset,
                            ap=[[D0, n], [D0 // 2, 2], [1, D0 // 2]]))

            # ---------------- out = x + F (flat layout) ----------------
            for g in range(NXC):
                xo = xpool.tile([P, FD], F32, tag="x", name="xo")
                nc.vector.tensor_add(
                    xo.rearrange("p (d k) -> p d k", k=KK),
                    x_res[:, g * FD:(g + 1) * FD]
                    .rearrange("p (d k) -> p d k", k=KK),
                    fperm[:, g * DQ:(g + 1) * DQ].unsqueeze(2)
                    .broadcast_to([P, DQ, KK]))
                eng = nc.scalar if g % 2 == 0 else nc.sync
                eng.dma_start(out=out_flat[:, g * FD:(g + 1) * FD], in_=xo)

    nc.compile()
    return nc


_CACHED = {}
_RUN_KWARGS = {}


def _get_program():
    if "nc" not in _CACHED:
        _CACHED["nc"] = build_program()
    return _CACHED["nc"]


def _bn_consts(b, gamma, beta, mean, var, nch):
    g = (gamma / np.sqrt(var + BN_EPS)).astype(np.float32)
    b2 = (beta - g * mean).astype(np.float32)
    def fold(v):
        return np.ascontiguousarray(np.asarray(v, np.float32).reshape(nch, P).T)
    return fold(b), fold(g), fold(b2)


def _swizzle(w, ncj, ncc):
    """w [J*128, C*128] -> out[j*128+p, c*128+ww] = w[j*128+ww, c*128+p]."""
    J, C = ncj, ncc
    a = np.asarray(w, np.float32).reshape(J, P, C, P)       # [j, ww, c, p]
    a = a.transpose(0, 3, 2, 1)                              # [j, p, c, ww]
    return np.ascontiguousarray(
        a.reshape(J * P, C * P).astype(ml_dtypes.bfloat16))


def kernel(**inputs):
    cfg = FULL
    B, n, m = cfg["B"], cfg["n"], cfg["m"]
    D0, C0, D1, D2, KK = cfg["D0"], cfg["C0"], cfg["D1"], cfg["D2"], cfg["KK"]

    x = np.asarray(inputs["x"], dtype=np.float32).reshape(B, n, D0, KK)
    xc = np.asarray(inputs["x_context"], dtype=np.float32)
    nvalid = np.asarray(inputs["num_valid_context_items"]).reshape(B).astype(np.int64)

    bf = ml_dtypes.bfloat16
    wq = np.ascontiguousarray(np.asarray(inputs["q_W"], np.float32).T.astype(bf))
    wks = _swizzle(inputs["k_W"], D1 // P, C0 // P)
    wvs = _swizzle(inputs["v_W"], D2 // P, C0 // P)
    wf = np.ascontiguousarray(np.asarray(inputs["f_W"], np.float32).T.astype(bf))

    qc = _bn_consts(inputs["q_b"], inputs["q_gamma"], inputs["q_beta"],
                    inputs["q_mean"], inputs["q_var"], D1 // P)
    kc = _bn_consts(inputs["k_b"], inputs["k_gamma"], inputs["k_beta"],
                    inputs["k_mean"], inputs["k_var"], D1 // P)
    vc = _bn_consts(inputs["v_b"], inputs["v_gamma"], inputs["v_beta"],
                    inputs["v_mean"], inputs["v_var"], D2 // P)
    fc = _bn_consts(inputs["f_b"], inputs["f_gamma"], inputs["f_beta"],
                    inputs["f_mean"], inputs["f_var"], D0 // P)

    ar = np.arange(m)
    in_maps = []
    for b in range(B):
        am = np.where(ar < nvalid[b], 0.0, NEG_MASK).astype(bf)
        in_maps.append({
            "xb": np.ascontiguousarray(x[b].astype(bf)),
            "xct": np.ascontiguousarray(xc[b].T.astype(bf)),
            "wq": wq, "wks": wks, "wvs": wvs, "wf": wf,
            "amask": am,
            "qcb": qc[0], "qcg": qc[1], "qc2": qc[2],
            "kcb": kc[0], "kcg": kc[1], "kc2": kc[2],
            "vcb": vc[0], "vcg": vc[1], "vc2": vc[2],
            "fcb": fc[0], "fcg": fc[1], "fc2": fc[2],
        })

    nc = _get_program()
    res = bass_utils.run_bass_kernel_spmd(nc, in_maps, core_ids=list(range(B)),
                                          **_RUN_KWARGS)
    _CACHED["last_results"] = res
    out = np.stack([r["out"] for r in res.results], axis=0)
    return out.reshape(B, n, D0, 7, 7).astype(np.float32)


# revision 4
# speedup vs baseline: 1.2609x; 1.0007x over previous
"""Trainium2 Bass kernel for nn_Attention_60155311948227 (sparse_attention) v2.

Data-parallel over batch B=8 across 8 NeuronCores (1 sample/core); weights
replicated. Host-side prep: weights cast to bf16 (strip-swizzled for K/V),
x_context transposed+cast to bf16 on host -> no on-chip XCT phase.

Per-core emission order (PE executes its queue in order):
  pool    A^T = sum_{7x7}(x) (f32 SWDGE loads, DVE reduce, PE transpose)
  K^T     j-loop: kp halves rotate PSUM A/D; BN drains lag-1; ksq -> kn2
          ones-matmuls accumulate in persistent PSUM (C=lo, B=hi)
  rk      sqrt -> DRAM scatter -> recip -> bcast [n, m] (gpsimd, overlapped)
  Q^T     8 strips, qps in B; BN; qn2 ones-matmuls in C -> rq
  S       accumulated per j into spA (A) / spB (D); mask+softmax (DVE/ACT)
  V^T     j-loop like K (vp B/C); per-j PE transposes (vtp D) -> v_nat
          [m(part), d2] (aliases kt); rv2 accumulated on DVE per j
  WV/F    fused per d2-chunk j (wvp C/B), lag-2 F matmuls into fps (A)
  F       BN; F^T transposes (ftp D); DRAM bounce -> fperm
  out     x chunks (sync HWDGE) + F broadcast in-place add, store (scalar)
"""

import sys

import numpy as np

try:
    import concourse.bacc as bacc
except ImportError:  # pragma: no cover
    sys.path.insert(0, "/opt/trn_rl_repo")
    import concourse.bacc as bacc

import ml_dtypes

import concourse.bass as bass
import concourse.tile as tile
from concourse import mybir
from concourse import bass_utils
from concourse.masks import make_identity

F32 = mybir.dt.float32
BF16 = mybir.dt.bfloat16
FP8 = mybir.dt.float8e4
DR = mybir.MatmulPerfMode.DoubleRow
VW_SCALE = 64.0
AF = mybir.ActivationFunctionType
ALU = mybir.AluOpType
AX = mybir.AxisListType

BN_EPS = 1e-5
NEG_MASK = -50.0
TEMP_INV = 100.0
NORM_EPS = 1e-24

FULL = dict(B=8, n=64, m=2048, D0=1024, C0=2048, D1=2048, D2=2048, KK=49)

P = 128


def build_program(cfg=None, num_devices=8):
    cfg = dict(FULL if cfg is None else cfg)
    n, m, D0, C0, D1, D2, KK = (
        cfg["n"], cfg["m"], cfg["D0"], cfg["C0"], cfg["D1"], cfg["D2"], cfg["KK"]
    )
    nc_d0, nc_c0, nc_d1, nc_d2, nc_m = D0 // P, C0 // P, D1 // P, D2 // P, m // P
    NT = 512
    mh = m // 2
    inv_kk = 1.0 / KK
    DQ = 32
    FD = DQ * KK                    # 1568 elements per flat chunk

    nc = bacc.Bacc("TRN2", target_bir_lowering=False, debug=False,
                   num_devices=num_devices)

    def din(name, shape, dt=F32):
        return nc.dram_tensor(name, shape, dt, kind="ExternalInput").ap()

    xb_in = din("xb", [n, D0, KK], BF16)             # bf16 x (pool + residual)
    xct_in = din("xct", [C0, m], BF16)               # host-transposed bf16
    xct8_in = din("xct8", [C0, m], FP8)              # fp8 copy for V proj
    wq_in = din("wq", [D0, D1], BF16)                # row strips
    wks_in = din("wks", [D1, C0], BF16)              # swizzled strips
    wvs_in = din("wvs", [D2, C0], FP8)               # swizzled strips, x64
    wf_in = din("wf", [D2, D0], BF16)                # row strips
    amask = din("amask", [m], BF16)
    qcb = din("qcb", [P, nc_d1]); qcg = din("qcg", [P, nc_d1]); qc2 = din("qc2", [P, nc_d1])
    kcb = din("kcb", [P, nc_d1]); kcg = din("kcg", [P, nc_d1]); kc2 = din("kc2", [P, nc_d1])
    vcb = din("vcb", [P, nc_d2]); vcg = din("vcg", [P, nc_d2]); vc2 = din("vc2", [P, nc_d2])
    fcb = din("fcb", [P, nc_d0]); fcg = din("fcg", [P, nc_d0]); fc2 = din("fc2", [P, nc_d0])
    out_d = nc.dram_tensor("out", [n, D0, KK], F32, kind="ExternalOutput").ap()
    xb_flat = xb_in.rearrange("nn d k -> (nn d k)").rearrange("(p f) -> p f", p=P)
    out_flat = out_d.rearrange("nn d k -> (nn d k)").rearrange("(p f) -> p f", p=P)
    FREE = xb_flat.shape[1]         # 25088
    NXC = FREE // FD                # 16 flat x chunks

    with tile.TileContext(nc) as tc:
        with (
            tc.tile_pool(name="consts", bufs=1) as consts,
            tc.tile_pool(name="bigmat", bufs=1) as bigmat,
            tc.tile_pool(name="wst", bufs=3) as wst,
            tc.tile_pool(name="vtjp", bufs=2) as vtjp,
            tc.tile_pool(name="scr", bufs=2) as scr,
            tc.tile_pool(name="smalls", bufs=2) as smalls,
            tc.tile_pool(name="xpool", bufs=3) as xpool,
            tc.tile_pool(name="ps", bufs=1, space="PSUM") as ps,
            tc.tile_pool(name="dscr", bufs=1, space="DRAM") as dscr,
        ):
            # ---------------- constants ----------------
            ident = consts.tile([P, P], BF16)
            make_identity(nc, ident)
            ident32 = consts.tile([P, P], F32)
            make_identity(nc, ident32)
            ones_col = consts.tile([P, 1], BF16)
            nc.vector.memset(ones_col, 1.0)
            eps_col = consts.tile([P, 1], F32)
            nc.vector.memset(eps_col, NORM_EPS)

            def cload(ap_in, nch):
                t = consts.tile([P, nch], F32, name=f"c_{ap_in.tensor.name}")
                nc.sync.dma_start(out=t, in_=ap_in)
                return t

            qcb_t = cload(qcb, nc_d1); qcg_t = cload(qcg, nc_d1); qc2_t = cload(qc2, nc_d1)
            kcb_t = cload(kcb, nc_d1); kcg_t = cload(kcg, nc_d1); kc2_t = cload(kc2, nc_d1)
            vcb_t = cload(vcb, nc_d2); vcg_t = cload(vcg, nc_d2); vc2_t = cload(vc2, nc_d2)
            fcb_t = cload(fcb, nc_d0); fcg_t = cload(fcg, nc_d0); fc2_t = cload(fc2, nc_d0)

            amask_bc = consts.tile([n, m], BF16, tag="amask_bc")
            nc.gpsimd.dma_start(
                out=amask_bc,
                in_=bass.AP(tensor=amask.tensor, offset=amask.offset,
                            ap=[[0, n]] + list(amask.ap)),
            )

            # ---------------- big SBUF tensors ----------------
            # xct as two separate m-half tiles so K's h=0 fills start after
            # only half the load; first two K strips hoisted
            xct_lo = bigmat.tile([P, nc_c0, mh], BF16, tag="xct_lo")
            xct_hi = bigmat.tile([P, nc_c0, mh], BF16, tag="xct_hi")
            kws0 = wst.tile([P, C0], BF16, tag="wst", name="kws0")
            nc.sync.dma_start(out=kws0, in_=wks_in[0:P, :])
            xct_r = xct_in.rearrange("(c p) m -> p c m", p=P)
            nc.sync.dma_start(out=xct_lo, in_=xct_r[:, :, :mh])
            kws1 = wst.tile([P, C0], BF16, tag="wst", name="kws1")
            nc.sync.dma_start(out=kws1, in_=wks_in[P:2 * P, :])
            nc.sync.dma_start(out=xct_hi, in_=xct_r[:, :, mh:])

            kt = bigmat.tile([P, nc_d1, m], BF16, tag="ktv", name="kt")

            # pooling state: loads+reduces interleaved into the K loop below
            # asum_all[p, g, dd] = sum_k x[p, (g*DQ+dd)*KK + k]
            asum_all = consts.tile([P, NXC, DQ], F32, name="asum_all")
            at = consts.tile([P, nc_d0, n], BF16)

            def pool_chunk(g):
                xt = xpool.tile([P, FD], BF16, tag="x", name="xt")
                nc.sync.dma_start(out=xt, in_=xb_flat[:, g * FD:(g + 1) * FD])
                nc.vector.reduce_sum(asum_all[:, g, :],
                                     xt.rearrange("p (d k) -> p d k", k=KK),
                                     axis=AX.X)

            def pool_finish():
                # PE transposes (post-K, cheap): at[pp, c, nn] = A^T[cP+pp, nn]
                for g in range(NXC):
                    atp = ps.tile([DQ, P], F32, tag="A", name="atp")
                    nc.tensor.transpose(atp, asum_all[:, g, :], ident32)
                    for half in range(2):
                        dglob = half * (D0 // 2) + g * DQ
                        base = dglob % P
                        nc.vector.tensor_copy(
                            out=at[base:base + DQ, dglob // P, :],
                            in_=atp[:, half::2])

            # ---------------- K^T projection with fused kn2 ----------------
            kn2_lo = ps.tile([1, mh], F32, tag="C", name="kn2_lo")
            kn2_hi = ps.tile([1, mh], F32, tag="B", name="kn2_hi")

            def k_fill(j, h, kws):
                kp = ps.tile([P, mh], F32, tag=("A" if h == 0 else "D"),
                             name="kp")
                xcth = xct_lo if h == 0 else xct_hi
                for c in range(nc_c0):
                    for nt in range(2):
                        nc.tensor.matmul(kp[:, nt * NT:(nt + 1) * NT],
                                         kws[:, c * P:(c + 1) * P],
                                         xcth[:, c, nt * NT:(nt + 1) * NT],
                                         start=(c == 0), stop=(c == nc_c0 - 1))
                return kp

            def k_drain(j, h, kp):
                ktj = kt[:, j, h * mh:(h + 1) * mh]
                nc.scalar.activation(ktj, kp, AF.Relu, bias=kcb_t[:, j:j + 1])
                nc.vector.tensor_scalar(out=ktj, in0=ktj,
                                        scalar1=kcg_t[:, j:j + 1],
                                        scalar2=kc2_t[:, j:j + 1],
                                        op0=ALU.mult, op1=ALU.add)
                ksq = scr.tile([P, mh], BF16, tag="ksq", name="ksq")
                nc.vector.tensor_mul(ksq, ktj, ktj)
                kn2 = kn2_lo if h == 0 else kn2_hi
                for nt in range(2):
                    nc.tensor.matmul(kn2[:, nt * NT:(nt + 1) * NT], ones_col,
                                     ksq[:, nt * NT:(nt + 1) * NT],
                                     start=(j == 0), stop=(j == nc_d1 - 1))

            prev = None
            for j in range(nc_d1):
                if j == 0:
                    kws = kws0
                elif j == 1:
                    kws = kws1
                else:
                    kws = wst.tile([P, C0], BF16, tag="wst", name="kws")
                    nc.sync.dma_start(out=kws, in_=wks_in[j * P:(j + 1) * P, :])
                if j < NXC // 2:
                    pool_chunk(2 * j)
                    pool_chunk(2 * j + 1)
                for h in range(2):
                    kp = k_fill(j, h, kws)
                    if prev is not None:
                        k_drain(*prev)
                    prev = (j, h, kp)
            k_drain(*prev)
            pool_finish()

            # rk chain: sqrt -> scatter [P, m/P] -> recip -> DRAM -> bcast
            # rk_row shares the rk_bc region (consumed before rk_bc is written)
            rk_row = consts.tile([1, m], F32, name="rk_row", tag="rk_bc")
            nc.scalar.activation(rk_row[:, :mh], kn2_lo, AF.Sqrt,
                                 bias=eps_col[:1, :])
            nc.scalar.activation(rk_row[:, mh:], kn2_hi, AF.Sqrt,
                                 bias=eps_col[:1, :])
            scr_k = dscr.tile([m], F32, name="scr_k")
            nc.gpsimd.dma_start(out=scr_k, in_=rk_row)
            rk128 = smalls.tile([P, nc_m], F32, name="rk128")
            nc.gpsimd.dma_start(out=rk128,
                                in_=bass.AP(tensor=scr_k.tensor, offset=scr_k.offset,
                                            ap=[[1, P], [P, nc_m]]))
            nc.vector.reciprocal(rk128, rk128)
            scr_k2 = dscr.tile([m], F32, name="scr_k2")
            nc.gpsimd.dma_start(
                out=bass.AP(tensor=scr_k2.tensor, offset=scr_k2.offset,
                            ap=[[1, P], [P, nc_m]]),
                in_=rk128)
            rk_bc = consts.tile([n, m], F32, name="rk_bc", tag="rk_bc")
            nc.gpsimd.dma_start(out=rk_bc,
                                in_=bass.AP(tensor=scr_k2.tensor, offset=scr_k2.offset,
                                            ap=[[0, n], [1, m]]))

            # ---------------- Q^T projection + qn2 ----------------
            qt = consts.tile([P, nc_d1, n], BF16)
            qps = ps.tile([P, nc_d1, n], F32, tag="B", name="qps")
            # fp8 xct for V proj reuses the xct_lo region (dead after K);
            # m-half DMAs interleave with the Q weight strips on sync
            xct8 = bigmat.tile([P, nc_c0, m], FP8, tag="xct_lo", name="xct8")
            xct8_r = xct8_in.rearrange("(c p) m -> p c m", p=P)
            jperz = max(1, 512 // n)
            for c in range(nc_d0):
                qwb = wst.tile([P, D1], BF16, tag="wst", name="qwb")
                nc.sync.dma_start(out=qwb, in_=wq_in[c * P:(c + 1) * P, :])
                if c == 1:
                    nc.sync.dma_start(out=xct8[:, :, :mh], in_=xct8_r[:, :, :mh])
                elif c == 2:
                    nc.sync.dma_start(out=xct8[:, :, mh:], in_=xct8_r[:, :, mh:])
                for j in range(nc_d1):
                    nc.tensor.matmul(qps[:, j, :], qwb[:, j * P:(j + 1) * P],
                                     at[:, c, :],
                                     start=(c == 0 and j % jperz == 0),
                                     stop=(c == nc_d0 - 1 and
                                           j % jperz == jperz - 1),
                                     skip_group_check=True)
            qn2 = ps.tile([1, n], F32, tag="C")
            for j in range(nc_d1):
                q1 = smalls.tile([P, n], BF16, name="q1")
                nc.scalar.activation(q1, qps[:, j, :], AF.Relu,
                                     bias=qcb_t[:, j:j + 1], scale=inv_kk)
                nc.vector.tensor_scalar(out=qt[:, j, :], in0=q1,
                                        scalar1=qcg_t[:, j:j + 1],
                                        scalar2=qc2_t[:, j:j + 1],
                                        op0=ALU.mult, op1=ALU.add)
                qsq = smalls.tile([P, n], BF16, name="qsq")
                nc.scalar.activation(qsq, qt[:, j, :], AF.Square)
                nc.tensor.matmul(qn2, ones_col, qsq,
                                 start=(j == 0), stop=(j == nc_d1 - 1))
            rq_row = smalls.tile([1, n], F32, name="rq_row")
            nc.scalar.activation(rq_row, qn2, AF.Sqrt, bias=eps_col[:1, :])
            scr_q = dscr.tile([n], F32, name="scr_q")
            nc.gpsimd.dma_start(out=scr_q, in_=rq_row)
            rq_col = smalls.tile([n, 1], F32, name="rq_col")
            nc.gpsimd.dma_start(out=rq_col,
                                in_=bass.AP(tensor=scr_q.tensor, offset=scr_q.offset,
                                            ap=[[1, n], [1, 1]]))
            nc.vector.reciprocal(rq_col, rq_col)

            # ---------------- S = Q K^T (accumulated per j) ----------------
            spA = ps.tile([n, mh], F32, tag="A", name="spA")
            spB = ps.tile([n, mh], F32, tag="D", name="spB")
            for j in range(nc_d1):
                for h in range(2):
                    sp = spA if h == 0 else spB
                    for nt in range(2):
                        lo = h * mh + nt * NT
                        nc.tensor.matmul(sp[:, nt * NT:(nt + 1) * NT],
                                         qt[:, j, :], kt[:, j, lo:lo + NT],
                                         start=(j == 0), stop=(j == nc_d1 - 1))

            # ---------------- softmax ----------------
            mx = smalls.tile([n, 1], F32, name="mx")
            mxb = smalls.tile([n, 1], F32, name="mxb")
            p_t = consts.tile([n, m], BF16, name="p_t", tag="amask_bc")
            pden = smalls.tile([n, 1], F32, name="pden")
            pdenb = smalls.tile([n, 1], F32, name="pdenb")
            for h in range(2):
                sp = spA if h == 0 else spB
                nc.vector.tensor_scalar(out=sp, in0=sp, scalar1=rq_col,
                                        scalar2=None, op0=ALU.mult)
                nc.vector.tensor_mul(sp, sp, rk_bc[:, h * mh:(h + 1) * mh])
                nc.vector.tensor_add(sp, sp, amask_bc[:, h * mh:(h + 1) * mh])
                nc.vector.tensor_reduce(mx if h == 0 else mxb, sp, axis=AX.X,
                                        op=ALU.max, negate=True)
            nc.vector.tensor_tensor(out=mx, in0=mx, in1=mxb, op=ALU.min)
            ebias = smalls.tile([n, 1], F32, name="ebias")
            nc.vector.tensor_scalar_mul(ebias, mx, TEMP_INV)
            nc.scalar.activation(p_t[:, :mh], spA, AF.Exp, bias=ebias,
                                 scale=TEMP_INV, accum_out=pden)
            nc.scalar.activation(p_t[:, mh:], spB, AF.Exp, bias=ebias,
                                 scale=TEMP_INV, accum_out=pdenb)
            nc.vector.tensor_add(pden, pden, pdenb)
            nc.vector.reciprocal(pden, pden)
            nc.vector.tensor_scalar_mul(p_t, p_t, pden)

            # ---------------- V^T -> v_nat with fused rv2 ----------------
            v_nat = bigmat.tile([P, nc_m, D2], BF16, tag="ktv", name="v_nat")
            rv2 = consts.tile([P, nc_m], F32, name="rv2")
            nc.vector.memset(rv2, NORM_EPS)
            pt_sb = consts.tile([P, nc_m, n], BF16)

            def v_fill(j, h, vws):
                # fp8 DoubleRow: two 128-deep contraction chunks per matmul
                vp = ps.tile([P, mh], F32, tag=("B" if h == 0 else "C"),
                             name="vp")
                ncp = nc_c0 // 2
                for cp in range(ncp):
                    wpair = vws[:, cp * 2 * P:(cp + 1) * 2 * P].rearrange(
                        "p (two w) -> p two w", two=2)
                    for nt in range(2):
                        lo = h * mh + nt * NT
                        nc.tensor.matmul(vp[:, nt * NT:(nt + 1) * NT],
                                         wpair,
                                         xct8[:, 2 * cp:2 * cp + 2, lo:lo + NT],
                                         start=(cp == 0), stop=(cp == ncp - 1),
                                         perf_mode=DR)
                return vp

            def v_bn(j, h, vp, vtj):
                vtjh = vtj[:, h * mh:(h + 1) * mh]
                nc.scalar.activation(vtjh, vp, AF.Relu, bias=vcb_t[:, j:j + 1],
                                     scale=1.0 / VW_SCALE)
                nc.vector.tensor_scalar(out=vtjh, in0=vtjh,
                                        scalar1=vcg_t[:, j:j + 1],
                                        scalar2=vc2_t[:, j:j + 1],
                                        op0=ALU.mult, op1=ALU.add)

            def v_store(j, vtj):
                vtp = ps.tile([P, nc_m, P], BF16, tag="D", name="vtp")
                for i in range(nc_m):
                    nc.tensor.transpose(vtp[:, i, :], vtj[:, i * P:(i + 1) * P],
                                        ident)
                vslab = v_nat[:, :, j * P:(j + 1) * P]
                nc.vector.tensor_copy(out=vslab, in_=vtp)
                vsq = scr.tile([P, nc_m, P], BF16, tag="vsq", name="vsq", bufs=1)
                nc.vector.tensor_mul(vsq, vslab, vslab)
                vred = smalls.tile([P, nc_m], F32, name="vred")
                nc.vector.reduce_sum(vred, vsq, axis=AX.X)
                nc.vector.tensor_add(rv2, rv2, vred)

            pv = None
            for j in range(nc_d2):
                vws = wst.tile([P, C0], FP8, tag="wst", name="vws")
                nc.sync.dma_start(out=vws, in_=wvs_in[j * P:(j + 1) * P, :])
                vp0 = v_fill(j, 0, vws)
                if j == 1:
                    # P^T transposes: p_t ready (softmax ran during j=0 fills)
                    ptp = ps.tile([P, nc_m, n], BF16, tag="C", name="ptp")
                    for i in range(nc_m):
                        nc.tensor.transpose(ptp[:, i, :],
                                            p_t[:, i * P:(i + 1) * P],
                                            ident[:n, :n])
                    nc.vector.tensor_copy(out=pt_sb, in_=ptp)
                if pv is not None:
                    v_store(*pv)
                vp1 = v_fill(j, 1, vws)
                vtj = vtjp.tile([P, m], BF16, tag="vtj", name="vtj")
                v_bn(j, 0, vp0, vtj)
                v_bn(j, 1, vp1, vtj)
                pv = (j, vtj)
            v_store(*pv)

            # x_res prefetch into the xct8/xct_hi regions (dead after V proj)
            FH = FREE // 2
            x_res_lo = bigmat.tile([P, FH], BF16, tag="xct_lo", name="x_res_lo")
            x_res_hi = bigmat.tile([P, FH], BF16, tag="xct_hi", name="x_res_hi")
            nc.gpsimd.dma_start(out=x_res_lo, in_=xb_flat[:, :FH])
            nc.gpsimd.dma_start(out=x_res_hi, in_=xb_flat[:, FH:])

            # rv = rsqrt(rv2); scale P^T rows (broadcast + single mul)
            rv = smalls.tile([P, nc_m], F32, name="rv")
            nc.scalar.activation(rv, rv2, AF.Sqrt)
            nc.vector.reciprocal(rv, rv)
            rv_bc = scr.tile([P, nc_m, n], F32, tag="vsq", name="rv_bc", bufs=1)
            nc.vector.tensor_copy(out=rv_bc,
                                  in_=rv.unsqueeze(2).broadcast_to([P, nc_m, n]))
            nc.vector.tensor_mul(pt_sb, pt_sb, rv_bc)

            # ------------- WV^T and F^T fused over d2 chunks -------------
            fps = ps.tile([P, nc_d0, n], F32, tag="A", name="fps")
            ddperz = max(1, 512 // n)
            fstrips = {}

            def f_mm(j, wvj):
                for dd in range(nc_d0):
                    nc.tensor.matmul(fps[:, dd, :],
                                     fstrips[j][:, dd * P:(dd + 1) * P],
                                     wvj,
                                     start=(j == 0 and dd % ddperz == 0),
                                     stop=(j == nc_d2 - 1 and
                                           dd % ddperz == ddperz - 1),
                                     skip_group_check=True)

            wv_state = []
            for j in range(nc_d2):
                fwb = wst.tile([P, D0], BF16, tag="wst", name="fwb")
                nc.sync.dma_start(out=fwb, in_=wf_in[j * P:(j + 1) * P, :])
                fstrips[j] = fwb
                wvp = ps.tile([P, n], F32, tag=("C" if j % 2 == 0 else "B"),
                              name="wvp")
                for i in range(nc_m):
                    nc.tensor.matmul(wvp, v_nat[:, i, j * P:(j + 1) * P],
                                     pt_sb[:, i, :],
                                     start=(i == 0), stop=(i == nc_m - 1))
                wvj = smalls.tile([P, n], BF16, name="wvj", tag="wvj", bufs=3)
                nc.vector.tensor_copy(out=wvj, in_=wvp)
                wv_state.append((j, wvj))
                if len(wv_state) >= 3:
                    f_mm(*wv_state.pop(0))
            while wv_state:
                f_mm(*wv_state.pop(0))

            # ---------------- F BN, F^T -> flat layout ----------------
            ft = consts.tile([P, nc_d0, n], F32)
            for dd in range(nc_d0):
                f1 = smalls.tile([P, n], F32, name="f1")
                nc.scalar.activation(f1, fps[:, dd, :], AF.Relu,
                                     bias=fcb_t[:, dd:dd + 1])
                nc.vector.tensor_scalar(out=ft[:, dd, :], in0=f1,
                                        scalar1=fcg_t[:, dd:dd + 1],
                                        scalar2=fc2_t[:, dd:dd + 1],
                                        op0=ALU.mult, op1=ALU.add)
            fnat = consts.tile([n, D0], F32, tag="rk_bc")
            for dd in range(nc_d0):
                ftp = ps.tile([n, P], F32, tag="D", name="ftp")
                nc.tensor.transpose(ftp, ft[:, dd, :], ident32)
                nc.vector.tensor_copy(out=fnat[:, dd * P:(dd + 1) * P], in_=ftp)
            f_scr = dscr.tile([n, D0], F32, name="f_scr")
            nc.scalar.dma_start(out=f_scr, in_=fnat)
            fperm = consts.tile([P, D0 // 2], BF16, name="fperm")
            nc.gpsimd.dma_start(
                out=fperm,
                in_=bass.AP(tensor=f_scr.tensor, offset=f_scr.offset,
                            ap=[[D0, n], [D0 // 2, 2], [1, D0 // 2]]))

            # ---------------- out = x + F (flat layout) ----------------
            # adds split ~2:1 vector/gpsimd; stores alternate scalar/sync
            NGH = NXC // 2
            for g in range(NXC):
                xo = xpool.tile([P, FD], F32, tag="x", name="xo")
                xr = x_res_lo if g < NGH else x_res_hi
                off = (g if g < NGH else g - NGH) * FD
                veng = nc.gpsimd if g % 3 == 2 else nc.vector
                veng.tensor_add(
                    xo.rearrange("p (d k) -> p d k", k=KK),
                    xr[:, off:off + FD].rearrange("p (d k) -> p d k", k=KK),
                    fperm[:, g * DQ:(g + 1) * DQ].unsqueeze(2)
                    .broadcast_to([P, DQ, KK]))
                eng = nc.scalar if g % 2 == 0 else nc.sync
                eng.dma_start(out=out_flat[:, g * FD:(g + 1) * FD], in_=xo)

    nc.compile()
    return nc


_CACHED = {}
_RUN_KWARGS = {}


def _get_program():
    if "nc" not in _CACHED:
        _CACHED["nc"] = build_program()
    return _CACHED["nc"]


def _bn_consts(b, gamma, beta, mean, var, nch):
    g = (gamma / np.sqrt(var + BN_EPS)).astype(np.float32)
    b2 = (beta - g * mean).astype(np.float32)
    def fold(v):
        return np.ascontiguousarray(np.asarray(v, np.float32).reshape(nch, P).T)
    return fold(b), fold(g), fold(b2)


def _swizzle(w, ncj, ncc, dtype=ml_dtypes.bfloat16, scale=1.0):
    """w [J*128, C*128] -> out[j*128+p, c*128+ww] = w[j*128+ww, c*128+p]."""
    J, C = ncj, ncc
    a = np.asarray(w, np.float32).reshape(J, P, C, P) * scale  # [j, ww, c, p]
    a = a.transpose(0, 3, 2, 1)                                # [j, p, c, ww]
    return np.ascontiguousarray(
        np.clip(a.reshape(J * P, C * P), -240.0, 240.0).astype(dtype))


def kernel(**inputs):
    cfg = FULL
    B, n, m = cfg["B"], cfg["n"], cfg["m"]
    D0, C0, D1, D2, KK = cfg["D0"], cfg["C0"], cfg["D1"], cfg["D2"], cfg["KK"]

    x = np.asarray(inputs["x"], dtype=np.float32).reshape(B, n, D0, KK)
    xc = np.asarray(inputs["x_context"], dtype=np.float32)
    nvalid = np.asarray(inputs["num_valid_context_items"]).reshape(B).astype(np.int64)

    bf = ml_dtypes.bfloat16
    f8 = ml_dtypes.float8_e4m3fn
    wq = np.ascontiguousarray(np.asarray(inputs["q_W"], np.float32).T.astype(bf))
    wks = _swizzle(inputs["k_W"], D1 // P, C0 // P)
    wvs = _swizzle(inputs["v_W"], D2 // P, C0 // P, dtype=f8, scale=VW_SCALE)
    wf = np.ascontiguousarray(np.asarray(inputs["f_W"], np.float32).T.astype(bf))

    qc = _bn_consts(inputs["q_b"], inputs["q_gamma"], inputs["q_beta"],
                    inputs["q_mean"], inputs["q_var"], D1 // P)
    kc = _bn_consts(inputs["k_b"], inputs["k_gamma"], inputs["k_beta"],
                    inputs["k_mean"], inputs["k_var"], D1 // P)
    vc = _bn_consts(inputs["v_b"], inputs["v_gamma"], inputs["v_beta"],
                    inputs["v_mean"], inputs["v_var"], D2 // P)
    fc = _bn_consts(inputs["f_b"], inputs["f_gamma"], inputs["f_beta"],
                    inputs["f_mean"], inputs["f_var"], D0 // P)

    ar = np.arange(m)
    in_maps = []
    for b in range(B):
        am = np.where(ar < nvalid[b], 0.0, NEG_MASK).astype(bf)
        xcT = np.ascontiguousarray(xc[b].T)
        in_maps.append({
            "xb": np.ascontiguousarray(x[b].astype(bf)),
            "xct": xcT.astype(bf),
            "xct8": np.clip(xcT, -240.0, 240.0).astype(f8),
            "wq": wq, "wks": wks, "wvs": wvs, "wf": wf,
            "amask": am,
            "qcb": qc[0], "qcg": qc[1], "qc2": qc[2],
            "kcb": kc[0], "kcg": kc[1], "kc2": kc[2],
            "vcb": vc[0], "vcg": vc[1], "vc2": vc[2],
            "fcb": fc[0], "fcg": fc[1], "fc2": fc[2],
        })

    nc = _get_program()
    res = bass_utils.run_bass_kernel_spmd(nc, in_maps, core_ids=list(range(B)),
                                          **_RUN_KWARGS)
    _CACHED["last_results"] = res
    out = np.stack([r["out"] for r in res.results], axis=0)
    return out.reshape(B, n, D0, 7, 7).astype(np.float32)


# revision 5
# speedup vs baseline: 1.2826x; 1.0172x over previous
"""Trainium2 Bass kernel for nn_Attention_60155311948227 (sparse_attention) v2.

Data-parallel over batch B=8 across 8 NeuronCores (1 sample/core); weights
replicated. Host-side prep: weights cast to bf16 (strip-swizzled for K/V),
x_context transposed+cast to bf16 on host -> no on-chip XCT phase.

Per-core emission order (PE executes its queue in order):
  pool    A^T = sum_{7x7}(x) (f32 SWDGE loads, DVE reduce, PE transpose)
  K^T     j-loop: kp halves rotate PSUM A/D; BN drains lag-1; ksq -> kn2
          ones-matmuls accumulate in persistent PSUM (C=lo, B=hi)
  rk      sqrt -> DRAM scatter -> recip -> bcast [n, m] (gpsimd, overlapped)
  Q^T     8 strips, qps in B; BN; qn2 ones-matmuls in C -> rq
  S       accumulated per j into spA (A) / spB (D); mask+softmax (DVE/ACT)
  V^T     j-loop like K (vp B/C); per-j PE transposes (vtp D) -> v_nat
          [m(part), d2] (aliases kt); rv2 accumulated on DVE per j
  WV/F    fused per d2-chunk j (wvp C/B), lag-2 F matmuls into fps (A)
  F       BN; F^T transposes (ftp D); DRAM bounce -> fperm
  out     x chunks (sync HWDGE) + F broadcast in-place add, store (scalar)
"""

import sys

import numpy as np

try:
    import concourse.bacc as bacc
except ImportError:  # pragma: no cover
    sys.path.insert(0, "/opt/trn_rl_repo")
    import concourse.bacc as bacc

import ml_dtypes

import concourse.bass as bass
import concourse.tile as tile
from concourse import mybir
from concourse import bass_utils
from concourse.masks import make_identity

F32 = mybir.dt.float32
BF16 = mybir.dt.bfloat16
FP8 = mybir.dt.float8e4
DR = mybir.MatmulPerfMode.DoubleRow
VW_SCALE = 64.0
AF = mybir.ActivationFunctionType
ALU = mybir.AluOpType
AX = mybir.AxisListType

BN_EPS = 1e-5
NEG_MASK = -50.0
TEMP_INV = 100.0
NORM_EPS = 1e-24

FULL = dict(B=8, n=64, m=2048, D0=1024, C0=2048, D1=2048, D2=2048, KK=49)

P = 128


def build_program(cfg=None, num_devices=8):
    cfg = dict(FULL if cfg is None else cfg)
    n, m, D0, C0, D1, D2, KK = (
        cfg["n"], cfg["m"], cfg["D0"], cfg["C0"], cfg["D1"], cfg["D2"], cfg["KK"]
    )
    nc_d0, nc_c0, nc_d1, nc_d2, nc_m = D0 // P, C0 // P, D1 // P, D2 // P, m // P
    NT = 512
    mh = m // 2
    inv_kk = 1.0 / KK
    DQ = 32
    FD = DQ * KK                    # 1568 elements per flat chunk

    nc = bacc.Bacc("TRN2", target_bir_lowering=False, debug=False,
                   num_devices=num_devices)

    def din(name, shape, dt=F32):
        return nc.dram_tensor(name, shape, dt, kind="ExternalInput").ap()

    xb_in = din("xb", [n, D0, KK], BF16)             # bf16 x (pool + residual)
    xct_in = din("xct", [C0, m], BF16)               # host-transposed bf16
    xct8_in = din("xct8", [C0, m], FP8)              # fp8 copy for V proj
    wq_in = din("wq", [D0, D1], BF16)                # row strips
    wks_in = din("wks", [D1, C0], BF16)              # swizzled strips
    wvs_in = din("wvs", [D2, C0], FP8)               # swizzled strips, x64
    wf_in = din("wf", [D2, D0], BF16)                # row strips
    amask = din("amask", [m], BF16)
    qcb = din("qcb", [P, nc_d1]); qcg = din("qcg", [P, nc_d1]); qc2 = din("qc2", [P, nc_d1])
    kcb = din("kcb", [P, nc_d1]); kcg = din("kcg", [P, nc_d1]); kc2 = din("kc2", [P, nc_d1])
    vcb = din("vcb", [P, nc_d2]); vcg = din("vcg", [P, nc_d2]); vc2 = din("vc2", [P, nc_d2])
    fcb = din("fcb", [P, nc_d0]); fcg = din("fcg", [P, nc_d0]); fc2 = din("fc2", [P, nc_d0])
    out_d = nc.dram_tensor("out", [n, D0, KK], F32, kind="ExternalOutput").ap()
    xb_flat = xb_in.rearrange("nn d k -> (nn d k)").rearrange("(p f) -> p f", p=P)
    out_flat = out_d.rearrange("nn d k -> (nn d k)").rearrange("(p f) -> p f", p=P)
    FREE = xb_flat.shape[1]         # 25088
    NXC = FREE // FD                # 16 flat x chunks

    with tile.TileContext(nc) as tc:
        with (
            tc.tile_pool(name="consts", bufs=1) as consts,
            tc.tile_pool(name="bigmat", bufs=1) as bigmat,
            tc.tile_pool(name="wst", bufs=3) as wst,
            tc.tile_pool(name="vtjp", bufs=2) as vtjp,
            tc.tile_pool(name="scr", bufs=2) as scr,
            tc.tile_pool(name="smalls", bufs=2) as smalls,
            tc.tile_pool(name="xpool", bufs=3) as xpool,
            tc.tile_pool(name="ps", bufs=1, space="PSUM") as ps,
            tc.tile_pool(name="dscr", bufs=1, space="DRAM") as dscr,
        ):
            # ---------------- constants ----------------
            ident = consts.tile([P, P], BF16)
            make_identity(nc, ident)
            ident32 = consts.tile([P, P], F32)
            make_identity(nc, ident32)
            ones_col = consts.tile([P, 1], BF16)
            nc.vector.memset(ones_col, 1.0)
            eps_col = consts.tile([P, 1], F32)
            nc.vector.memset(eps_col, NORM_EPS)

            def cload(ap_in, nch):
                # gpsimd queue: keeps the sync FIFO clear for xct/strips
                t = consts.tile([P, nch], F32, name=f"c_{ap_in.tensor.name}")
                nc.gpsimd.dma_start(out=t, in_=ap_in)
                return t

            qcb_t = cload(qcb, nc_d1); qcg_t = cload(qcg, nc_d1); qc2_t = cload(qc2, nc_d1)
            kcb_t = cload(kcb, nc_d1); kcg_t = cload(kcg, nc_d1); kc2_t = cload(kc2, nc_d1)
            vcb_t = cload(vcb, nc_d2); vcg_t = cload(vcg, nc_d2); vc2_t = cload(vc2, nc_d2)
            fcb_t = cload(fcb, nc_d0); fcg_t = cload(fcg, nc_d0); fc2_t = cload(fc2, nc_d0)

            amask_bc = consts.tile([n, m], BF16, tag="amask_bc")
            nc.gpsimd.dma_start(
                out=amask_bc,
                in_=bass.AP(tensor=amask.tensor, offset=amask.offset,
                            ap=[[0, n]] + list(amask.ap)),
            )

            # ---------------- big SBUF tensors ----------------
            # xct as two separate m-half tiles so K's h=0 fills start after
            # only half the load; first two K strips hoisted
            xct_lo = bigmat.tile([P, nc_c0, mh], BF16, tag="xct_lo")
            xct_hi = bigmat.tile([P, nc_c0, mh], BF16, tag="xct_hi")
            kws0 = wst.tile([P, C0], BF16, tag="wst", name="kws0")
            nc.sync.dma_start(out=kws0, in_=wks_in[0:P, :])
            xct_r = xct_in.rearrange("(c p) m -> p c m", p=P)
            nc.sync.dma_start(out=xct_lo, in_=xct_r[:, :, :mh])
            kws1 = wst.tile([P, C0], BF16, tag="wst", name="kws1")
            nc.sync.dma_start(out=kws1, in_=wks_in[P:2 * P, :])
            nc.sync.dma_start(out=xct_hi, in_=xct_r[:, :, mh:])

            kt = bigmat.tile([P, nc_d1, m], BF16, tag="ktv", name="kt")

            # pooling state: loads+reduces interleaved into the K loop below
            # asum_all[p, g, dd] = sum_k x[p, (g*DQ+dd)*KK + k]
            asum_all = consts.tile([P, NXC, DQ], F32, name="asum_all")
            at = consts.tile([P, nc_d0, n], BF16)

            def pool_chunk(g):
                xt = xpool.tile([P, FD], BF16, tag="x", name="xt")
                nc.sync.dma_start(out=xt, in_=xb_flat[:, g * FD:(g + 1) * FD])
                nc.vector.reduce_sum(asum_all[:, g, :],
                                     xt.rearrange("p (d k) -> p d k", k=KK),
                                     axis=AX.X)

            def pool_finish():
                # PE transposes (post-K, cheap): at[pp, c, nn] = A^T[cP+pp, nn]
                for g in range(NXC):
                    atp = ps.tile([DQ, P], F32, tag="A", name="atp")
                    nc.tensor.transpose(atp, asum_all[:, g, :], ident32)
                    for half in range(2):
                        dglob = half * (D0 // 2) + g * DQ
                        base = dglob % P
                        nc.vector.tensor_copy(
                            out=at[base:base + DQ, dglob // P, :],
                            in_=atp[:, half::2])

            # ---------------- K^T projection with fused kn2 ----------------
            kn2_lo = ps.tile([1, mh], F32, tag="C", name="kn2_lo")
            kn2_hi = ps.tile([1, mh], F32, tag="B", name="kn2_hi")

            def k_fill(j, h, kws):
                kp = ps.tile([P, mh], F32, tag=("A" if h == 0 else "D"),
                             name="kp")
                xcth = xct_lo if h == 0 else xct_hi
                for c in range(nc_c0):
                    for nt in range(2):
                        nc.tensor.matmul(kp[:, nt * NT:(nt + 1) * NT],
                                         kws[:, c * P:(c + 1) * P],
                                         xcth[:, c, nt * NT:(nt + 1) * NT],
                                         start=(c == 0), stop=(c == nc_c0 - 1))
                return kp

            def k_drain(j, h, kp):
                ktj = kt[:, j, h * mh:(h + 1) * mh]
                nc.scalar.activation(ktj, kp, AF.Relu, bias=kcb_t[:, j:j + 1])
                nc.vector.tensor_scalar(out=ktj, in0=ktj,
                                        scalar1=kcg_t[:, j:j + 1],
                                        scalar2=kc2_t[:, j:j + 1],
                                        op0=ALU.mult, op1=ALU.add)
                ksq = scr.tile([P, mh], BF16, tag="ksq", name="ksq")
                nc.vector.tensor_mul(ksq, ktj, ktj)
                kn2 = kn2_lo if h == 0 else kn2_hi
                for nt in range(2):
                    nc.tensor.matmul(kn2[:, nt * NT:(nt + 1) * NT], ones_col,
                                     ksq[:, nt * NT:(nt + 1) * NT],
                                     start=(j == 0), stop=(j == nc_d1 - 1))

            prev = None
            for j in range(nc_d1):
                if j == 0:
                    kws = kws0
                elif j == 1:
                    kws = kws1
                else:
                    kws = wst.tile([P, C0], BF16, tag="wst", name="kws")
                    nc.sync.dma_start(out=kws, in_=wks_in[j * P:(j + 1) * P, :])
                if j < NXC // 2:
                    pool_chunk(2 * j)
                    pool_chunk(2 * j + 1)
                for h in range(2):
                    kp = k_fill(j, h, kws)
                    if prev is not None:
                        k_drain(*prev)
                    prev = (j, h, kp)
            pool_finish()
            k_drain(*prev)

            # rk chain: sqrt -> scatter [P, m/P] -> recip -> DRAM -> bcast
            # rk_row shares the rk_bc region (consumed before rk_bc is written)
            rk_row = consts.tile([1, m], F32, name="rk_row", tag="rk_bc")
            nc.scalar.activation(rk_row[:, :mh], kn2_lo, AF.Sqrt,
                                 bias=eps_col[:1, :])
            nc.scalar.activation(rk_row[:, mh:], kn2_hi, AF.Sqrt,
                                 bias=eps_col[:1, :])
            scr_k = dscr.tile([m], F32, name="scr_k")
            nc.gpsimd.dma_start(out=scr_k, in_=rk_row)
            rk128 = smalls.tile([P, nc_m], F32, name="rk128")
            nc.gpsimd.dma_start(out=rk128,
                                in_=bass.AP(tensor=scr_k.tensor, offset=scr_k.offset,
                                            ap=[[1, P], [P, nc_m]]))
            nc.vector.reciprocal(rk128, rk128)
            scr_k2 = dscr.tile([m], F32, name="scr_k2")
            nc.gpsimd.dma_start(
                out=bass.AP(tensor=scr_k2.tensor, offset=scr_k2.offset,
                            ap=[[1, P], [P, nc_m]]),
                in_=rk128)
            rk_bc = consts.tile([n, m], F32, name="rk_bc", tag="rk_bc")
            nc.gpsimd.dma_start(out=rk_bc,
                                in_=bass.AP(tensor=scr_k2.tensor, offset=scr_k2.offset,
                                            ap=[[0, n], [1, m]]))

            # ---------------- Q^T projection + qn2 ----------------
            qt = consts.tile([P, nc_d1, n], BF16)
            qps = ps.tile([P, nc_d1, n], F32, tag="B", name="qps")
            # fp8 xct for V proj reuses the xct_lo region (dead after K);
            # m-half DMAs interleave with the Q weight strips on sync
            xct8 = bigmat.tile([P, nc_c0, m], FP8, tag="xct_lo", name="xct8")
            xct8_r = xct8_in.rearrange("(c p) m -> p c m", p=P)
            jperz = max(1, 512 // n)
            for c in range(nc_d0):
                qwb = wst.tile([P, D1], BF16, tag="wst", name="qwb")
                nc.sync.dma_start(out=qwb, in_=wq_in[c * P:(c + 1) * P, :])
                if c == 1:
                    nc.sync.dma_start(out=xct8[:, :, :mh], in_=xct8_r[:, :, :mh])
                elif c == 2:
                    nc.sync.dma_start(out=xct8[:, :, mh:], in_=xct8_r[:, :, mh:])
                for j in range(nc_d1):
                    nc.tensor.matmul(qps[:, j, :], qwb[:, j * P:(j + 1) * P],
                                     at[:, c, :],
                                     start=(c == 0 and j % jperz == 0),
                                     stop=(c == nc_d0 - 1 and
                                           j % jperz == jperz - 1),
                                     skip_group_check=True)
            # Q BN chain first (ACT/DVE only), then S (PE waits only on qt),
            # then qn2 last (qsq long since ready -> no PE stall)
            qn2 = ps.tile([1, n], F32, tag="C")
            spA = ps.tile([n, mh], F32, tag="A", name="spA")
            spB = ps.tile([n, mh], F32, tag="D", name="spB")
            qsqs = []
            for j in range(nc_d1):
                q1 = smalls.tile([P, n], BF16, name="q1")
                nc.scalar.activation(q1, qps[:, j, :], AF.Relu,
                                     bias=qcb_t[:, j:j + 1], scale=inv_kk)
                nc.vector.tensor_scalar(out=qt[:, j, :], in0=q1,
                                        scalar1=qcg_t[:, j:j + 1],
                                        scalar2=qc2_t[:, j:j + 1],
                                        op0=ALU.mult, op1=ALU.add)
                qsq = smalls.tile([P, n], BF16, name="qsq", tag="qsq", bufs=16)
                nc.scalar.activation(qsq, qt[:, j, :], AF.Square)
                qsqs.append(qsq)
            for j in range(nc_d1):
                for h in range(2):
                    sp = spA if h == 0 else spB
                    kth = kt[:, j, h * mh:(h + 1) * mh]
                    for nt in range(2):
                        nc.tensor.matmul(sp[:, nt * NT:(nt + 1) * NT],
                                         qt[:, j, :],
                                         kth[:, nt * NT:(nt + 1) * NT],
                                         start=(j == 0), stop=(j == nc_d1 - 1))
            for j in range(nc_d1):
                nc.tensor.matmul(qn2, ones_col, qsqs[j],
                                 start=(j == 0), stop=(j == nc_d1 - 1))
            rq_row = smalls.tile([1, n], F32, name="rq_row")
            nc.scalar.activation(rq_row, qn2, AF.Sqrt, bias=eps_col[:1, :])
            scr_q = dscr.tile([n], F32, name="scr_q")
            nc.gpsimd.dma_start(out=scr_q, in_=rq_row)
            rq_col = smalls.tile([n, 1], F32, name="rq_col")
            nc.gpsimd.dma_start(out=rq_col,
                                in_=bass.AP(tensor=scr_q.tensor, offset=scr_q.offset,
                                            ap=[[1, n], [1, 1]]))
            nc.vector.reciprocal(rq_col, rq_col)

            # ---------------- softmax ----------------
            mx = smalls.tile([n, 1], F32, name="mx")
            mxb = smalls.tile([n, 1], F32, name="mxb")
            p_t = consts.tile([n, m], BF16, name="p_t", tag="amask_bc")
            pden = smalls.tile([n, 1], F32, name="pden")
            pdenb = smalls.tile([n, 1], F32, name="pdenb")
            for h in range(2):
                sp = spA if h == 0 else spB
                nc.vector.tensor_scalar(out=sp, in0=sp, scalar1=rq_col,
                                        scalar2=None, op0=ALU.mult)
                nc.vector.tensor_mul(sp, sp, rk_bc[:, h * mh:(h + 1) * mh])
                nc.vector.tensor_add(sp, sp, amask_bc[:, h * mh:(h + 1) * mh])
                nc.vector.tensor_reduce(mx if h == 0 else mxb, sp, axis=AX.X,
                                        op=ALU.max, negate=True)
            nc.vector.tensor_tensor(out=mx, in0=mx, in1=mxb, op=ALU.min)
            ebias = smalls.tile([n, 1], F32, name="ebias")
            nc.vector.tensor_scalar_mul(ebias, mx, TEMP_INV)
            nc.scalar.activation(p_t[:, :mh], spA, AF.Exp, bias=ebias,
                                 scale=TEMP_INV, accum_out=pden)
            nc.scalar.activation(p_t[:, mh:], spB, AF.Exp, bias=ebias,
                                 scale=TEMP_INV, accum_out=pdenb)
            nc.vector.tensor_add(pden, pden, pdenb)
            nc.vector.reciprocal(pden, pden)
            nc.vector.tensor_scalar_mul(p_t, p_t, pden)

            # ---------------- V^T -> v_nat with fused rv2 ----------------
            v_nat = bigmat.tile([P, nc_m, D2], BF16, tag="ktv", name="v_nat")
            rv2 = consts.tile([P, nc_m], F32, name="rv2")
            nc.vector.memset(rv2, NORM_EPS)
            pt_sb = consts.tile([P, nc_m, n], BF16)

            def v_fill(j, h, vws):
                # fp8 DoubleRow: two 128-deep contraction chunks per matmul
                vp = ps.tile([P, mh], F32, tag=("B" if h == 0 else "C"),
                             name="vp")
                ncp = nc_c0 // 2
                for cp in range(ncp):
                    wpair = vws[:, cp * 2 * P:(cp + 1) * 2 * P].rearrange(
                        "p (two w) -> p two w", two=2)
                    for nt in range(2):
                        lo = h * mh + nt * NT
                        nc.tensor.matmul(vp[:, nt * NT:(nt + 1) * NT],
                                         wpair,
                                         xct8[:, 2 * cp:2 * cp + 2, lo:lo + NT],
                                         start=(cp == 0), stop=(cp == ncp - 1),
                                         perf_mode=DR)
                return vp

            def v_bn(j, h, vp, vtj):
                vtjh = vtj[:, h * mh:(h + 1) * mh]
                nc.scalar.activation(vtjh, vp, AF.Relu, bias=vcb_t[:, j:j + 1],
                                     scale=1.0 / VW_SCALE)
                nc.vector.tensor_scalar(out=vtjh, in0=vtjh,
                                        scalar1=vcg_t[:, j:j + 1],
                                        scalar2=vc2_t[:, j:j + 1],
                                        op0=ALU.mult, op1=ALU.add)

            def v_store(j, vtj):
                vtp = ps.tile([P, nc_m, P], BF16, tag="D", name="vtp")
                for i in range(nc_m):
                    nc.tensor.transpose(vtp[:, i, :], vtj[:, i * P:(i + 1) * P],
                                        ident)
                vslab = v_nat[:, :, j * P:(j + 1) * P]
                nc.vector.tensor_copy(out=vslab, in_=vtp)
                vsq = scr.tile([P, nc_m, P], BF16, tag="vsq", name="vsq", bufs=1)
                nc.vector.tensor_mul(vsq, vslab, vslab)
                vred = smalls.tile([P, nc_m], F32, name="vred")
                nc.vector.reduce_sum(vred, vsq, axis=AX.X)
                nc.vector.tensor_add(rv2, rv2, vred)

            pv = None
            for j in range(nc_d2):
                vws = wst.tile([P, C0], FP8, tag="wst", name="vws")
                nc.sync.dma_start(out=vws, in_=wvs_in[j * P:(j + 1) * P, :])
                vp0 = v_fill(j, 0, vws)
                if j == 1:
                    # P^T transposes: p_t ready (softmax ran during j=0 fills)
                    ptp = ps.tile([P, nc_m, n], BF16, tag="C", name="ptp")
                    for i in range(nc_m):
                        nc.tensor.transpose(ptp[:, i, :],
                                            p_t[:, i * P:(i + 1) * P],
                                            ident[:n, :n])
                    nc.vector.tensor_copy(out=pt_sb, in_=ptp)
                if pv is not None:
                    v_store(*pv)
                vp1 = v_fill(j, 1, vws)
                vtj = vtjp.tile([P, m], BF16, tag="vtj", name="vtj")
                v_bn(j, 0, vp0, vtj)
                v_bn(j, 1, vp1, vtj)
                pv = (j, vtj)
            v_store(*pv)

            # x_res prefetch into the xct8/xct_hi regions (dead after V proj)
            FH = FREE // 2
            x_res_lo = bigmat.tile([P, FH], BF16, tag="xct_lo", name="x_res_lo")
            x_res_hi = bigmat.tile([P, FH], BF16, tag="xct_hi", name="x_res_hi")
            nc.gpsimd.dma_start(out=x_res_lo, in_=xb_flat[:, :FH])
            nc.gpsimd.dma_start(out=x_res_hi, in_=xb_flat[:, FH:])

            # rv = rsqrt(rv2); scale P^T rows (broadcast + single mul)
            rv = smalls.tile([P, nc_m], F32, name="rv")
            nc.scalar.activation(rv, rv2, AF.Sqrt)
            nc.vector.reciprocal(rv, rv)
            rv_bc = scr.tile([P, nc_m, n], F32, tag="vsq", name="rv_bc", bufs=1)
            nc.vector.tensor_copy(out=rv_bc,
                                  in_=rv.unsqueeze(2).broadcast_to([P, nc_m, n]))
            nc.vector.tensor_mul(pt_sb, pt_sb, rv_bc)

            # ------------- WV^T and F^T fused over d2 chunks -------------
            fps = ps.tile([P, nc_d0, n], F32, tag="A", name="fps")
            ddperz = max(1, 512 // n)
            fstrips = {}

            def f_mm(j, wvj):
                for dd in range(nc_d0):
                    nc.tensor.matmul(fps[:, dd, :],
                                     fstrips[j][:, dd * P:(dd + 1) * P],
                                     wvj,
                                     start=(j == 0 and dd % ddperz == 0),
                                     stop=(j == nc_d2 - 1 and
                                           dd % ddperz == ddperz - 1),
                                     skip_group_check=True)

            wv_state = []
            for j in range(nc_d2):
                fwb = wst.tile([P, D0], BF16, tag="wst", name="fwb")
                nc.sync.dma_start(out=fwb, in_=wf_in[j * P:(j + 1) * P, :])
                fstrips[j] = fwb
                wvp = ps.tile([P, n], F32, tag=("C" if j % 2 == 0 else "B"),
                              name="wvp")
                for i in range(nc_m):
                    nc.tensor.matmul(wvp, v_nat[:, i, j * P:(j + 1) * P],
                                     pt_sb[:, i, :],
                                     start=(i == 0), stop=(i == nc_m - 1))
                wvj = smalls.tile([P, n], BF16, name="wvj", tag="wvj", bufs=3)
                nc.vector.tensor_copy(out=wvj, in_=wvp)
                wv_state.append((j, wvj))
                if len(wv_state) >= 3:
                    f_mm(*wv_state.pop(0))
            while wv_state:
                f_mm(*wv_state.pop(0))

            # ---------------- F BN, F^T -> flat layout ----------------
            ft = consts.tile([P, nc_d0, n], F32)
            for dd in range(nc_d0):
                f1 = smalls.tile([P, n], F32, name="f1")
                nc.scalar.activation(f1, fps[:, dd, :], AF.Relu,
                                     bias=fcb_t[:, dd:dd + 1])
                nc.vector.tensor_scalar(out=ft[:, dd, :], in0=f1,
                                        scalar1=fcg_t[:, dd:dd + 1],
                                        scalar2=fc2_t[:, dd:dd + 1],
                                        op0=ALU.mult, op1=ALU.add)
            fnat = consts.tile([n, D0], F32, tag="rk_bc")
            for dd in range(nc_d0):
                ftp = ps.tile([n, P], F32, tag="D", name="ftp")
                nc.tensor.transpose(ftp, ft[:, dd, :], ident32)
                nc.vector.tensor_copy(out=fnat[:, dd * P:(dd + 1) * P], in_=ftp)
            f_scr = dscr.tile([n, D0], F32, name="f_scr")
            nc.scalar.dma_start(out=f_scr, in_=fnat)
            fperm = consts.tile([P, D0 // 2], BF16, name="fperm")
            nc.gpsimd.dma_start(
                out=fperm,
                in_=bass.AP(tensor=f_scr.tensor, offset=f_scr.offset,
                            ap=[[D0, n], [D0 // 2, 2], [1, D0 // 2]]))

            # ---------------- out = x + F (flat layout) ----------------
            # adds split ~2:1 vector/gpsimd; stores alternate scalar/sync
            NGH = NXC // 2
            for g in range(NXC):
                xo = xpool.tile([P, FD], F32, tag="x", name="xo")
                xr = x_res_lo if g < NGH else x_res_hi
                off = (g if g < NGH else g - NGH) * FD
                veng = nc.gpsimd if g % 3 == 2 else nc.vector
                veng.tensor_add(
                    xo.rearrange("p (d k) -> p d k", k=KK),
                    xr[:, off:off + FD].rearrange("p (d k) -> p d k", k=KK),
                    fperm[:, g * DQ:(g + 1) * DQ].unsqueeze(2)
                    .broadcast_to([P, DQ, KK]))
                eng = nc.scalar if g % 2 == 0 else nc.sync
                eng.dma_start(out=out_flat[:, g * FD:(g + 1) * FD], in_=xo)

    nc.compile()
    return nc


_CACHED = {}
_RUN_KWARGS = {}


def _get_program():
    if "nc" not in _CACHED:
        _CACHED["nc"] = build_program()
    return _CACHED["nc"]


def _bn_consts(b, gamma, beta, mean, var, nch):
    g = (gamma / np.sqrt(var + BN_EPS)).astype(np.float32)
    b2 = (beta - g * mean).astype(np.float32)
    def fold(v):
        return np.ascontiguousarray(np.asarray(v, np.float32).reshape(nch, P).T)
    return fold(b), fold(g), fold(b2)


def _swizzle(w, ncj, ncc, dtype=ml_dtypes.bfloat16, scale=1.0):
    """w [J*128, C*128] -> out[j*128+p, c*128+ww] = w[j*128+ww, c*128+p]."""
    J, C = ncj, ncc
    a = np.asarray(w, np.float32).reshape(J, P, C, P) * scale  # [j, ww, c, p]
    a = a.transpose(0, 3, 2, 1)                                # [j, p, c, ww]
    return np.ascontiguousarray(
        np.clip(a.reshape(J * P, C * P), -240.0, 240.0).astype(dtype))


def kernel(**inputs):
    cfg = FULL
    B, n, m = cfg["B"], cfg["n"], cfg["m"]
    D0, C0, D1, D2, KK = cfg["D0"], cfg["C0"], cfg["D1"], cfg["D2"], cfg["KK"]

    x = np.asarray(inputs["x"], dtype=np.float32).reshape(B, n, D0, KK)
    xc = np.asarray(inputs["x_context"], dtype=np.float32)
    nvalid = np.asarray(inputs["num_valid_context_items"]).reshape(B).astype(np.int64)

    bf = ml_dtypes.bfloat16
    f8 = ml_dtypes.float8_e4m3fn
    wq = np.ascontiguousarray(np.asarray(inputs["q_W"], np.float32).T.astype(bf))
    wks = _swizzle(inputs["k_W"], D1 // P, C0 // P)
    wvs = _swizzle(inputs["v_W"], D2 // P, C0 // P, dtype=f8, scale=VW_SCALE)
    wf = np.ascontiguousarray(np.asarray(inputs["f_W"], np.float32).T.astype(bf))

    qc = _bn_consts(inputs["q_b"], inputs["q_gamma"], inputs["q_beta"],
                    inputs["q_mean"], inputs["q_var"], D1 // P)
    kc = _bn_consts(inputs["k_b"], inputs["k_gamma"], inputs["k_beta"],
                    inputs["k_mean"], inputs["k_var"], D1 // P)
    vc = _bn_consts(inputs["v_b"], inputs["v_gamma"], inputs["v_beta"],
                    inputs["v_mean"], inputs["v_var"], D2 // P)
    fc = _bn_consts(inputs["f_b"], inputs["f_gamma"], inputs["f_beta"],
                    inputs["f_mean"], inputs["f_var"], D0 // P)

    ar = np.arange(m)
    in_maps = []
    for b in range(B):
        am = np.where(ar < nvalid[b], 0.0, NEG_MASK).astype(bf)
        xcT = np.ascontiguousarray(xc[b].T)
        in_maps.append({
            "xb": np.ascontiguousarray(x[b].astype(bf)),
            "xct": xcT.astype(bf),
            "xct8": np.clip(xcT, -240.0, 240.0).astype(f8),
            "wq": wq, "wks": wks, "wvs": wvs, "wf": wf,
            "amask": am,
            "qcb": qc[0], "qcg": qc[1], "qc2": qc[2],
            "kcb": kc[0], "kcg": kc[1], "kc2": kc[2],
            "vcb": vc[0], "vcg": vc[1], "vc2": vc[2],
            "fcb": fc[0], "fcg": fc[1], "fc2": fc[2],
        })

    nc = _get_program()
    res = bass_utils.run_bass_kernel_spmd(nc, in_maps, core_ids=list(range(B)),
                                          **_RUN_KWARGS)
    _CACHED["last_results"] = res
    out = np.stack([r["out"] for r in res.results], axis=0)
    return out.reshape(B, n, D0, 7, 7).astype(np.float32)


# revision 6
# speedup vs baseline: 1.2865x; 1.0031x over previous
"""Trainium2 Bass kernel for nn_Attention_60155311948227 (sparse_attention) v2.

Data-parallel over batch B=8 across 8 NeuronCores (1 sample/core); weights
replicated. Host-side prep: weights cast to bf16 (strip-swizzled for K/V),
x_context transposed+cast to bf16 on host -> no on-chip XCT phase.

Per-core emission order (PE executes its queue in order):
  pool    A^T = sum_{7x7}(x) (f32 SWDGE loads, DVE reduce, PE transpose)
  K^T     j-loop: kp halves rotate PSUM A/D; BN drains lag-1; ksq -> kn2
          ones-matmuls accumulate in persistent PSUM (C=lo, B=hi)
  rk      sqrt -> DRAM scatter -> recip -> bcast [n, m] (gpsimd, overlapped)
  Q^T     8 strips, qps in B; BN; qn2 ones-matmuls in C -> rq
  S       accumulated per j into spA (A) / spB (D); mask+softmax (DVE/ACT)
  V^T     j-loop like K (vp B/C); per-j PE transposes (vtp D) -> v_nat
          [m(part), d2] (aliases kt); rv2 accumulated on DVE per j
  WV/F    fused per d2-chunk j (wvp C/B), lag-2 F matmuls into fps (A)
  F       BN; F^T transposes (ftp D); DRAM bounce -> fperm
  out     x chunks (sync HWDGE) + F broadcast in-place add, store (scalar)
"""

import sys

import numpy as np

try:
    import concourse.bacc as bacc
except ImportError:  # pragma: no cover
    sys.path.insert(0, "/opt/trn_rl_repo")
    import concourse.bacc as bacc

import ml_dtypes

import concourse.bass as bass
import concourse.tile as tile
from concourse import mybir
from concourse import bass_utils
from concourse.masks import make_identity

F32 = mybir.dt.float32
BF16 = mybir.dt.bfloat16
FP8 = mybir.dt.float8e4
DR = mybir.MatmulPerfMode.DoubleRow
VW_SCALE = 64.0
AF = mybir.ActivationFunctionType
ALU = mybir.AluOpType
AX = mybir.AxisListType

BN_EPS = 1e-5
NEG_MASK = -50.0
TEMP_INV = 100.0
NORM_EPS = 1e-24

FULL = dict(B=8, n=64, m=2048, D0=1024, C0=2048, D1=2048, D2=2048, KK=49)

P = 128


def build_program(cfg=None, num_devices=8):
    cfg = dict(FULL if cfg is None else cfg)
    n, m, D0, C0, D1, D2, KK = (
        cfg["n"], cfg["m"], cfg["D0"], cfg["C0"], cfg["D1"], cfg["D2"], cfg["KK"]
    )
    nc_d0, nc_c0, nc_d1, nc_d2, nc_m = D0 // P, C0 // P, D1 // P, D2 // P, m // P
    NT = 512
    mh = m // 2
    inv_kk = 1.0 / KK
    DQ = 32
    FD = DQ * KK                    # 1568 elements per flat chunk

    nc = bacc.Bacc("TRN2", target_bir_lowering=False, debug=False,
                   num_devices=num_devices)

    def din(name, shape, dt=F32):
        return nc.dram_tensor(name, shape, dt, kind="ExternalInput").ap()

    xb_in = din("xb", [n, D0, KK], BF16)             # bf16 x (pool + residual)
    xct_in = din("xct", [C0, m], BF16)               # host-transposed bf16
    xct8_in = din("xct8", [C0, m], FP8)              # fp8 copy for V proj
    wq_in = din("wq", [D0, D1], BF16)                # row strips
    wks_in = din("wks", [D1, C0], BF16)              # swizzled strips
    wvs_in = din("wvs", [D2, C0], FP8)               # swizzled strips, x64
    wf_in = din("wf", [D2, D0], BF16)                # row strips
    amask = din("amask", [m], BF16)
    qcb = din("qcb", [P, nc_d1]); qcg = din("qcg", [P, nc_d1]); qc2 = din("qc2", [P, nc_d1])
    kcb = din("kcb", [P, nc_d1]); kcg = din("kcg", [P, nc_d1]); kc2 = din("kc2", [P, nc_d1])
    vcb = din("vcb", [P, nc_d2]); vcg = din("vcg", [P, nc_d2]); vc2 = din("vc2", [P, nc_d2])
    fcb = din("fcb", [P, nc_d0]); fcg = din("fcg", [P, nc_d0]); fc2 = din("fc2", [P, nc_d0])
    out_d = nc.dram_tensor("out", [n, D0, KK], F32, kind="ExternalOutput").ap()
    xb_flat = xb_in.rearrange("nn d k -> (nn d k)").rearrange("(p f) -> p f", p=P)
    out_flat = out_d.rearrange("nn d k -> (nn d k)").rearrange("(p f) -> p f", p=P)
    FREE = xb_flat.shape[1]         # 25088
    NXC = FREE // FD                # 16 flat x chunks

    with tile.TileContext(nc) as tc:
        with (
            tc.tile_pool(name="consts", bufs=1) as consts,
            tc.tile_pool(name="bigmat", bufs=1) as bigmat,
            tc.tile_pool(name="wst", bufs=3) as wst,
            tc.tile_pool(name="vtjp", bufs=2) as vtjp,
            tc.tile_pool(name="scr", bufs=2) as scr,
            tc.tile_pool(name="smalls", bufs=2) as smalls,
            tc.tile_pool(name="xpool", bufs=3) as xpool,
            tc.tile_pool(name="ps", bufs=1, space="PSUM") as ps,
            tc.tile_pool(name="dscr", bufs=1, space="DRAM") as dscr,
        ):
            # ---------------- constants ----------------
            ident = consts.tile([P, P], BF16)
            make_identity(nc, ident)
            ident32 = consts.tile([P, P], F32)
            make_identity(nc, ident32)
            ones_col = consts.tile([P, 1], BF16)
            nc.vector.memset(ones_col, 1.0)
            eps_col = consts.tile([P, 1], F32)
            nc.vector.memset(eps_col, NORM_EPS)

            def cload(ap_in, nch):
                # gpsimd queue: keeps the sync FIFO clear for xct/strips
                t = consts.tile([P, nch], F32, name=f"c_{ap_in.tensor.name}")
                nc.gpsimd.dma_start(out=t, in_=ap_in)
                return t

            qcb_t = cload(qcb, nc_d1); qcg_t = cload(qcg, nc_d1); qc2_t = cload(qc2, nc_d1)
            kcb_t = cload(kcb, nc_d1); kcg_t = cload(kcg, nc_d1); kc2_t = cload(kc2, nc_d1)
            vcb_t = cload(vcb, nc_d2); vcg_t = cload(vcg, nc_d2); vc2_t = cload(vc2, nc_d2)
            fcb_t = cload(fcb, nc_d0); fcg_t = cload(fcg, nc_d0); fc2_t = cload(fc2, nc_d0)

            amask_bc = consts.tile([n, m], BF16, tag="amask_bc")
            nc.gpsimd.dma_start(
                out=amask_bc,
                in_=bass.AP(tensor=amask.tensor, offset=amask.offset,
                            ap=[[0, n]] + list(amask.ap)),
            )

            # ---------------- big SBUF tensors ----------------
            # xct as two separate m-half tiles so K's h=0 fills start after
            # only half the load; first two K strips hoisted
            # first half as two quarter tiles so K's first fills start after
            # only 2.1MB of load
            xq0 = bigmat.tile([P, nc_c0, NT], BF16, tag="xq0")
            xq1 = bigmat.tile([P, nc_c0, NT], BF16, tag="xq1")
            xct_hi = bigmat.tile([P, nc_c0, mh], BF16, tag="xct_hi")
            kws0 = wst.tile([P, C0], BF16, tag="wst", name="kws0")
            nc.sync.dma_start(out=kws0, in_=wks_in[0:P, :])
            xct_r = xct_in.rearrange("(c p) m -> p c m", p=P)
            nc.sync.dma_start(out=xq0, in_=xct_r[:, :, :NT])
            kws1 = wst.tile([P, C0], BF16, tag="wst", name="kws1")
            nc.sync.dma_start(out=kws1, in_=wks_in[P:2 * P, :])
            nc.sync.dma_start(out=xq1, in_=xct_r[:, :, NT:mh])
            nc.sync.dma_start(out=xct_hi, in_=xct_r[:, :, mh:])

            kt = bigmat.tile([P, nc_d1, m], BF16, tag="ktv", name="kt")

            # pooling state: loads+reduces interleaved into the K loop below
            # asum_all[p, g, dd] = sum_k x[p, (g*DQ+dd)*KK + k]
            asum_all = consts.tile([P, NXC, DQ], F32, name="asum_all")
            at = consts.tile([P, nc_d0, n], BF16)

            def pool_chunk(g):
                xt = xpool.tile([P, FD], BF16, tag="x", name="xt")
                nc.sync.dma_start(out=xt, in_=xb_flat[:, g * FD:(g + 1) * FD])
                nc.vector.reduce_sum(asum_all[:, g, :],
                                     xt.rearrange("p (d k) -> p d k", k=KK),
                                     axis=AX.X)

            def pool_finish():
                # PE transposes (post-K, cheap): at[pp, c, nn] = A^T[cP+pp, nn]
                for g in range(NXC):
                    atp = ps.tile([DQ, P], F32, tag="A", name="atp")
                    nc.tensor.transpose(atp, asum_all[:, g, :], ident32)
                    for half in range(2):
                        dglob = half * (D0 // 2) + g * DQ
                        base = dglob % P
                        nc.vector.tensor_copy(
                            out=at[base:base + DQ, dglob // P, :],
                            in_=atp[:, half::2])

            # ---------------- K^T projection with fused kn2 ----------------
            kn2_lo = ps.tile([1, mh], F32, tag="C", name="kn2_lo")
            kn2_hi = ps.tile([1, mh], F32, tag="B", name="kn2_hi")

            def k_fill(j, h, kws):
                # nt-outer: the first 16 matmuls need only the first quarter
                kp = ps.tile([P, mh], F32, tag=("A" if h == 0 else "D"),
                             name="kp")
                for nt in range(2):
                    for c in range(nc_c0):
                        if h == 0:
                            mv = (xq0 if nt == 0 else xq1)[:, c, :]
                        else:
                            mv = xct_hi[:, c, nt * NT:(nt + 1) * NT]
                        nc.tensor.matmul(kp[:, nt * NT:(nt + 1) * NT],
                                         kws[:, c * P:(c + 1) * P], mv,
                                         start=(c == 0), stop=(c == nc_c0 - 1))
                return kp

            def k_drain(j, h, kp):
                ktj = kt[:, j, h * mh:(h + 1) * mh]
                nc.scalar.activation(ktj, kp, AF.Relu, bias=kcb_t[:, j:j + 1])
                nc.vector.tensor_scalar(out=ktj, in0=ktj,
                                        scalar1=kcg_t[:, j:j + 1],
                                        scalar2=kc2_t[:, j:j + 1],
                                        op0=ALU.mult, op1=ALU.add)
                ksq = scr.tile([P, mh], BF16, tag="ksq", name="ksq")
                nc.vector.tensor_mul(ksq, ktj, ktj)
                kn2 = kn2_lo if h == 0 else kn2_hi
                for nt in range(2):
                    nc.tensor.matmul(kn2[:, nt * NT:(nt + 1) * NT], ones_col,
                                     ksq[:, nt * NT:(nt + 1) * NT],
                                     start=(j == 0), stop=(j == nc_d1 - 1))

            prev = None
            for j in range(nc_d1):
                if j == 0:
                    kws = kws0
                elif j == 1:
                    kws = kws1
                else:
                    kws = wst.tile([P, C0], BF16, tag="wst", name="kws")
                    nc.sync.dma_start(out=kws, in_=wks_in[j * P:(j + 1) * P, :])
                if j < NXC // 2:
                    pool_chunk(2 * j)
                    pool_chunk(2 * j + 1)
                for h in range(2):
                    kp = k_fill(j, h, kws)
                    if prev is not None:
                        k_drain(*prev)
                    prev = (j, h, kp)
            pool_finish()
            k_drain(*prev)

            # rk chain: sqrt -> scatter [P, m/P] -> recip -> DRAM -> bcast
            # rk_row shares the rk_bc region (consumed before rk_bc is written)
            rk_row = consts.tile([1, m], F32, name="rk_row", tag="rk_bc")
            nc.scalar.activation(rk_row[:, :mh], kn2_lo, AF.Sqrt,
                                 bias=eps_col[:1, :])
            nc.scalar.activation(rk_row[:, mh:], kn2_hi, AF.Sqrt,
                                 bias=eps_col[:1, :])
            scr_k = dscr.tile([m], F32, name="scr_k")
            nc.gpsimd.dma_start(out=scr_k, in_=rk_row)
            rk128 = smalls.tile([P, nc_m], F32, name="rk128")
            nc.gpsimd.dma_start(out=rk128,
                                in_=bass.AP(tensor=scr_k.tensor, offset=scr_k.offset,
                                            ap=[[1, P], [P, nc_m]]))
            nc.vector.reciprocal(rk128, rk128)
            scr_k2 = dscr.tile([m], F32, name="scr_k2")
            nc.gpsimd.dma_start(
                out=bass.AP(tensor=scr_k2.tensor, offset=scr_k2.offset,
                            ap=[[1, P], [P, nc_m]]),
                in_=rk128)
            rk_bc = consts.tile([n, m], F32, name="rk_bc", tag="rk_bc")
            nc.gpsimd.dma_start(out=rk_bc,
                                in_=bass.AP(tensor=scr_k2.tensor, offset=scr_k2.offset,
                                            ap=[[0, n], [1, m]]))

            # ---------------- Q^T projection + qn2 ----------------
            qt = consts.tile([P, nc_d1, n], BF16)
            qps = ps.tile([P, nc_d1, n], F32, tag="B", name="qps")
            # fp8 xct for V proj reuses the freed quarter regions (dead after
            # K h=0); m-half DMAs interleave with the Q weight strips on sync
            xct8_lo = bigmat.tile([P, nc_c0, mh], FP8, tag="xq0", name="xct8_lo")
            xct8_hi = bigmat.tile([P, nc_c0, mh], FP8, tag="xq1", name="xct8_hi")
            xct8_r = xct8_in.rearrange("(c p) m -> p c m", p=P)
            jperz = max(1, 512 // n)
            for c in range(nc_d0):
                qwb = wst.tile([P, D1], BF16, tag="wst", name="qwb")
                nc.sync.dma_start(out=qwb, in_=wq_in[c * P:(c + 1) * P, :])
                if c == 1:
                    nc.sync.dma_start(out=xct8_lo, in_=xct8_r[:, :, :mh])
                elif c == 2:
                    nc.sync.dma_start(out=xct8_hi, in_=xct8_r[:, :, mh:])
                for j in range(nc_d1):
                    nc.tensor.matmul(qps[:, j, :], qwb[:, j * P:(j + 1) * P],
                                     at[:, c, :],
                                     start=(c == 0 and j % jperz == 0),
                                     stop=(c == nc_d0 - 1 and
                                           j % jperz == jperz - 1),
                                     skip_group_check=True)
            # Q BN chain first (ACT/DVE only), then S (PE waits only on qt),
            # then qn2 last (qsq long since ready -> no PE stall)
            qn2 = ps.tile([1, n], F32, tag="C")
            spA = ps.tile([n, mh], F32, tag="A", name="spA")
            spB = ps.tile([n, mh], F32, tag="D", name="spB")
            qsqs = []
            for j in range(nc_d1):
                q1 = smalls.tile([P, n], BF16, name="q1")
                nc.scalar.activation(q1, qps[:, j, :], AF.Relu,
                                     bias=qcb_t[:, j:j + 1], scale=inv_kk)
                nc.vector.tensor_scalar(out=qt[:, j, :], in0=q1,
                                        scalar1=qcg_t[:, j:j + 1],
                                        scalar2=qc2_t[:, j:j + 1],
                                        op0=ALU.mult, op1=ALU.add)
                qsq = smalls.tile([P, n], BF16, name="qsq", tag="qsq", bufs=16)
                nc.scalar.activation(qsq, qt[:, j, :], AF.Square)
                qsqs.append(qsq)
            for j in range(nc_d1):
                for h in range(2):
                    sp = spA if h == 0 else spB
                    kth = kt[:, j, h * mh:(h + 1) * mh]
                    for nt in range(2):
                        nc.tensor.matmul(sp[:, nt * NT:(nt + 1) * NT],
                                         qt[:, j, :],
                                         kth[:, nt * NT:(nt + 1) * NT],
                                         start=(j == 0), stop=(j == nc_d1 - 1))
            for j in range(nc_d1):
                nc.tensor.matmul(qn2, ones_col, qsqs[j],
                                 start=(j == 0), stop=(j == nc_d1 - 1))
            rq_row = smalls.tile([1, n], F32, name="rq_row")
            nc.scalar.activation(rq_row, qn2, AF.Sqrt, bias=eps_col[:1, :])
            scr_q = dscr.tile([n], F32, name="scr_q")
            nc.gpsimd.dma_start(out=scr_q, in_=rq_row)
            rq_col = smalls.tile([n, 1], F32, name="rq_col")
            nc.gpsimd.dma_start(out=rq_col,
                                in_=bass.AP(tensor=scr_q.tensor, offset=scr_q.offset,
                                            ap=[[1, n], [1, 1]]))
            nc.vector.reciprocal(rq_col, rq_col)

            # ---------------- softmax ----------------
            mx = smalls.tile([n, 1], F32, name="mx")
            mxb = smalls.tile([n, 1], F32, name="mxb")
            p_t = consts.tile([n, m], BF16, name="p_t", tag="amask_bc")
            pden = smalls.tile([n, 1], F32, name="pden")
            pdenb = smalls.tile([n, 1], F32, name="pdenb")
            for h in range(2):
                sp = spA if h == 0 else spB
                nc.vector.tensor_scalar(out=sp, in0=sp, scalar1=rq_col,
                                        scalar2=None, op0=ALU.mult)
                nc.vector.tensor_mul(sp, sp, rk_bc[:, h * mh:(h + 1) * mh])
                nc.vector.tensor_add(sp, sp, amask_bc[:, h * mh:(h + 1) * mh])
                nc.vector.tensor_reduce(mx if h == 0 else mxb, sp, axis=AX.X,
                                        op=ALU.max, negate=True)
            nc.vector.tensor_tensor(out=mx, in0=mx, in1=mxb, op=ALU.min)
            ebias = smalls.tile([n, 1], F32, name="ebias")
            nc.vector.tensor_scalar_mul(ebias, mx, TEMP_INV)
            nc.scalar.activation(p_t[:, :mh], spA, AF.Exp, bias=ebias,
                                 scale=TEMP_INV, accum_out=pden)
            nc.scalar.activation(p_t[:, mh:], spB, AF.Exp, bias=ebias,
                                 scale=TEMP_INV, accum_out=pdenb)
            nc.vector.tensor_add(pden, pden, pdenb)
            nc.vector.reciprocal(pden, pden)
            nc.vector.tensor_scalar_mul(p_t, p_t, pden)

            # ---------------- V^T -> v_nat with fused rv2 ----------------
            v_nat = bigmat.tile([P, nc_m, D2], BF16, tag="ktv", name="v_nat")
            rv2 = consts.tile([P, nc_m], F32, name="rv2")
            nc.vector.memset(rv2, NORM_EPS)
            pt_sb = consts.tile([P, nc_m, n], BF16)

            def v_fill(j, h, vws):
                # fp8 DoubleRow: two 128-deep contraction chunks per matmul
                vp = ps.tile([P, mh], F32, tag=("B" if h == 0 else "C"),
                             name="vp")
                ncp = nc_c0 // 2
                xc8 = xct8_lo if h == 0 else xct8_hi
                for cp in range(ncp):
                    wpair = vws[:, cp * 2 * P:(cp + 1) * 2 * P].rearrange(
                        "p (two w) -> p two w", two=2)
                    for nt in range(2):
                        nc.tensor.matmul(vp[:, nt * NT:(nt + 1) * NT],
                                         wpair,
                                         xc8[:, 2 * cp:2 * cp + 2,
                                             nt * NT:(nt + 1) * NT],
                                         start=(cp == 0), stop=(cp == ncp - 1),
                                         perf_mode=DR)
                return vp

            def v_bn(j, h, vp, vtj):
                vtjh = vtj[:, h * mh:(h + 1) * mh]
                nc.scalar.activation(vtjh, vp, AF.Relu, bias=vcb_t[:, j:j + 1],
                                     scale=1.0 / VW_SCALE)
                nc.vector.tensor_scalar(out=vtjh, in0=vtjh,
                                        scalar1=vcg_t[:, j:j + 1],
                                        scalar2=vc2_t[:, j:j + 1],
                                        op0=ALU.mult, op1=ALU.add)

            def v_store(j, vtj):
                vtp = ps.tile([P, nc_m, P], BF16, tag="D", name="vtp")
                for i in range(nc_m):
                    nc.tensor.transpose(vtp[:, i, :], vtj[:, i * P:(i + 1) * P],
                                        ident)
                vslab = v_nat[:, :, j * P:(j + 1) * P]
                nc.vector.tensor_copy(out=vslab, in_=vtp)
                vsq = scr.tile([P, nc_m, P], BF16, tag="vsq", name="vsq", bufs=1)
                nc.vector.tensor_mul(vsq, vslab, vslab)
                vred = smalls.tile([P, nc_m], F32, name="vred")
                nc.vector.reduce_sum(vred, vsq, axis=AX.X)
                nc.vector.tensor_add(rv2, rv2, vred)

            pv = None
            for j in range(nc_d2):
                vws = wst.tile([P, C0], FP8, tag="wst", name="vws")
                nc.sync.dma_start(out=vws, in_=wvs_in[j * P:(j + 1) * P, :])
                vp0 = v_fill(j, 0, vws)
                if j == 1:
                    # P^T transposes: p_t ready (softmax ran during j=0 fills)
                    ptp = ps.tile([P, nc_m, n], BF16, tag="C", name="ptp")
                    for i in range(nc_m):
                        nc.tensor.transpose(ptp[:, i, :],
                                            p_t[:, i * P:(i + 1) * P],
                                            ident[:n, :n])
                    nc.vector.tensor_copy(out=pt_sb, in_=ptp)
                if pv is not None:
                    v_store(*pv)
                vp1 = v_fill(j, 1, vws)
                vtj = vtjp.tile([P, m], BF16, tag="vtj", name="vtj")
                v_bn(j, 0, vp0, vtj)
                v_bn(j, 1, vp1, vtj)
                pv = (j, vtj)
            v_store(*pv)

            # x_res prefetch into the xct8/xct_hi regions (dead after V proj)
            FQ = FREE // 4
            x_res_q0 = bigmat.tile([P, FQ], BF16, tag="xq0", name="x_res_q0")
            x_res_q1 = bigmat.tile([P, FQ], BF16, tag="xq1", name="x_res_q1")
            x_res_hi = bigmat.tile([P, 2 * FQ], BF16, tag="xct_hi",
                                   name="x_res_hi")
            nc.gpsimd.dma_start(out=x_res_q0, in_=xb_flat[:, :FQ])
            nc.gpsimd.dma_start(out=x_res_q1, in_=xb_flat[:, FQ:2 * FQ])
            nc.gpsimd.dma_start(out=x_res_hi, in_=xb_flat[:, 2 * FQ:])

            # rv = rsqrt(rv2); scale P^T rows (broadcast + single mul)
            rv = smalls.tile([P, nc_m], F32, name="rv")
            nc.scalar.activation(rv, rv2, AF.Sqrt)
            nc.vector.reciprocal(rv, rv)
            rv_bc = scr.tile([P, nc_m, n], F32, tag="vsq", name="rv_bc", bufs=1)
            nc.vector.tensor_copy(out=rv_bc,
                                  in_=rv.unsqueeze(2).broadcast_to([P, nc_m, n]))
            nc.vector.tensor_mul(pt_sb, pt_sb, rv_bc)

            # ------------- WV^T and F^T fused over d2 chunks -------------
            fps = ps.tile([P, nc_d0, n], F32, tag="A", name="fps")
            ddperz = max(1, 512 // n)
            fstrips = {}

            def f_mm(j, wvj):
                for dd in range(nc_d0):
                    nc.tensor.matmul(fps[:, dd, :],
                                     fstrips[j][:, dd * P:(dd + 1) * P],
                                     wvj,
                                     start=(j == 0 and dd % ddperz == 0),
                                     stop=(j == nc_d2 - 1 and
                                           dd % ddperz == ddperz - 1),
                                     skip_group_check=True)

            wv_state = []
            for j in range(nc_d2):
                fwb = wst.tile([P, D0], BF16, tag="wst", name="fwb")
                nc.sync.dma_start(out=fwb, in_=wf_in[j * P:(j + 1) * P, :])
                fstrips[j] = fwb
                wvp = ps.tile([P, n], F32, tag=("C" if j % 2 == 0 else "B"),
                              name="wvp")
                for i in range(nc_m):
                    nc.tensor.matmul(wvp, v_nat[:, i, j * P:(j + 1) * P],
                                     pt_sb[:, i, :],
                                     start=(i == 0), stop=(i == nc_m - 1))
                wvj = smalls.tile([P, n], BF16, name="wvj", tag="wvj", bufs=3)
                nc.vector.tensor_copy(out=wvj, in_=wvp)
                wv_state.append((j, wvj))
                if len(wv_state) >= 3:
                    f_mm(*wv_state.pop(0))
            while wv_state:
                f_mm(*wv_state.pop(0))

            # ---------------- F BN, F^T -> flat layout ----------------
            ft = consts.tile([P, nc_d0, n], F32)
            for dd in range(nc_d0):
                f1 = smalls.tile([P, n], F32, name="f1")
                nc.scalar.activation(f1, fps[:, dd, :], AF.Relu,
                                     bias=fcb_t[:, dd:dd + 1])
                nc.vector.tensor_scalar(out=ft[:, dd, :], in0=f1,
                                        scalar1=fcg_t[:, dd:dd + 1],
                                        scalar2=fc2_t[:, dd:dd + 1],
                                        op0=ALU.mult, op1=ALU.add)
            fnat = consts.tile([n, D0], F32, tag="rk_bc")
            for dd in range(nc_d0):
                ftp = ps.tile([n, P], F32, tag="D", name="ftp")
                nc.tensor.transpose(ftp, ft[:, dd, :], ident32)
                nc.vector.tensor_copy(out=fnat[:, dd * P:(dd + 1) * P], in_=ftp)
            f_scr = dscr.tile([n, D0], F32, name="f_scr")
            nc.scalar.dma_start(out=f_scr, in_=fnat)
            fperm = consts.tile([P, D0 // 2], BF16, name="fperm")
            nc.gpsimd.dma_start(
                out=fperm,
                in_=bass.AP(tensor=f_scr.tensor, offset=f_scr.offset,
                            ap=[[D0, n], [D0 // 2, 2], [1, D0 // 2]]))

            # ---------------- out = x + F (flat layout) ----------------
            # adds split ~2:1 vector/gpsimd; stores alternate scalar/sync
            NGQ = NXC // 4
            for g in range(NXC):
                xo = xpool.tile([P, FD], F32, tag="x", name="xo")
                if g < NGQ:
                    xr, off = x_res_q0, g * FD
                elif g < 2 * NGQ:
                    xr, off = x_res_q1, (g - NGQ) * FD
                else:
                    xr, off = x_res_hi, (g - 2 * NGQ) * FD
                veng = nc.gpsimd if g % 3 == 2 else nc.vector
                veng.tensor_add(
                    xo.rearrange("p (d k) -> p d k", k=KK),
                    xr[:, off:off + FD].rearrange("p (d k) -> p d k", k=KK),
                    fperm[:, g * DQ:(g + 1) * DQ].unsqueeze(2)
                    .broadcast_to([P, DQ, KK]))
                eng = nc.scalar if g % 2 == 0 else nc.sync
                eng.dma_start(out=out_flat[:, g * FD:(g + 1) * FD], in_=xo)

    nc.compile()
    return nc


_CACHED = {}
_RUN_KWARGS = {}


def _get_program():
    if "nc" not in _CACHED:
        _CACHED["nc"] = build_program()
    return _CACHED["nc"]


def _bn_consts(b, gamma, beta, mean, var, nch):
    g = (gamma / np.sqrt(var + BN_EPS)).astype(np.float32)
    b2 = (beta - g * mean).astype(np.float32)
    def fold(v):
        return np.ascontiguousarray(np.asarray(v, np.float32).reshape(nch, P).T)
    return fold(b), fold(g), fold(b2)


def _swizzle(w, ncj, ncc, dtype=ml_dtypes.bfloat16, scale=1.0):
    """w [J*128, C*128] -> out[j*128+p, c*128+ww] = w[j*128+ww, c*128+p]."""
    J, C = ncj, ncc
    a = np.asarray(w, np.float32).reshape(J, P, C, P) * scale  # [j, ww, c, p]
    a = a.transpose(0, 3, 2, 1)                                # [j, p, c, ww]
    return np.ascontiguousarray(
        np.clip(a.reshape(J * P, C * P), -240.0, 240.0).astype(dtype))


def kernel(**inputs):
    cfg = FULL
    B, n, m = cfg["B"], cfg["n"], cfg["m"]
    D0, C0, D1, D2, KK = cfg["D0"], cfg["C0"], cfg["D1"], cfg["D2"], cfg["KK"]

    x = np.asarray(inputs["x"], dtype=np.float32).reshape(B, n, D0, KK)
    xc = np.asarray(inputs["x_context"], dtype=np.float32)
    nvalid = np.asarray(inputs["num_valid_context_items"]).reshape(B).astype(np.int64)

    bf = ml_dtypes.bfloat16
    f8 = ml_dtypes.float8_e4m3fn
    wq = np.ascontiguousarray(np.asarray(inputs["q_W"], np.float32).T.astype(bf))
    wks = _swizzle(inputs["k_W"], D1 // P, C0 // P)
    wvs = _swizzle(inputs["v_W"], D2 // P, C0 // P, dtype=f8, scale=VW_SCALE)
    wf = np.ascontiguousarray(np.asarray(inputs["f_W"], np.float32).T.astype(bf))

    qc = _bn_consts(inputs["q_b"], inputs["q_gamma"], inputs["q_beta"],
                    inputs["q_mean"], inputs["q_var"], D1 // P)
    kc = _bn_consts(inputs["k_b"], inputs["k_gamma"], inputs["k_beta"],
                    inputs["k_mean"], inputs["k_var"], D1 // P)
    vc = _bn_consts(inputs["v_b"], inputs["v_gamma"], inputs["v_beta"],
                    inputs["v_mean"], inputs["v_var"], D2 // P)
    fc = _bn_consts(inputs["f_b"], inputs["f_gamma"], inputs["f_beta"],
                    inputs["f_mean"], inputs["f_var"], D0 // P)

    ar = np.arange(m)
    in_maps = []
    for b in range(B):
        am = np.where(ar < nvalid[b], 0.0, NEG_MASK).astype(bf)
        xcT = np.ascontiguousarray(xc[b].T)
        in_maps.append({
            "xb": np.ascontiguousarray(x[b].astype(bf)),
            "xct": xcT.astype(bf),
            "xct8": np.clip(xcT, -240.0, 240.0).astype(f8),
            "wq": wq, "wks": wks, "wvs": wvs, "wf": wf,
            "amask": am,
            "qcb": qc[0], "qcg": qc[1], "qc2": qc[2],
            "kcb": kc[0], "kcg": kc[1], "kc2": kc[2],
            "vcb": vc[0], "vcg": vc[1], "vc2": vc[2],
            "fcb": fc[0], "fcg": fc[1], "fc2": fc[2],
        })

    nc = _get_program()
    res = bass_utils.run_bass_kernel_spmd(nc, in_maps, core_ids=list(range(B)),
                                          **_RUN_KWARGS)
    _CACHED["last_results"] = res
    out = np.stack([r["out"] for r in res.results], axis=0)
    return out.reshape(B, n, D0, 7, 7).astype(np.float32)
